# revision 21
# baseline (speedup 1.0000x reference)
"""DenoiseLSTM Trainium2 kernel (8 NeuronCores, SPMD), v2.

Structure (single fused program per core; recurrences replicated, vocab
projection sharded over V=32000 -> 4000 cols/core):

  A. weight loads + token gathers + h0 inits.
  B. encoder (fwd+bwd interleaved). Input projections are computed into
     SBUF ring chunks (no DRAM roundtrip) and PRELOADED into the gate
     PSUM via an identity matmul, so the recurrent matmuls accumulate on
     top (start=False) and no vector add sits on the critical path. The
     final gate multiply writes h straight into mem_T (no copy); the
     next step's matmul reads it back strided.
  C. decoder, same gate scheme, h written into an H ring chunk.
     Attention + FFN + vocab projection for each completed 32-step
     t-chunk are emitted into the gaps between decoder steps via a work
     queue, so the big parallel matmuls fill the PE idle time of the
     serial recurrence. Logits are written bf16; the host upcasts.
"""
import sys

sys.path.insert(0, "/opt/trn_rl_repo")

from contextlib import ExitStack

import numpy as np
import ml_dtypes

import concourse.bass as bass
import concourse.bacc as bacc
import concourse.mybir as mybir
import concourse.tile as tile
from concourse.bass_utils import run_bass_kernel_spmd
from concourse.masks import make_identity

bf16 = ml_dtypes.bfloat16
F32 = mybir.dt.float32
BF16 = mybir.dt.bfloat16
I16 = mybir.dt.int16
AF = mybir.ActivationFunctionType
ALU = mybir.AluOpType
AX = mybir.AxisListType

B = 32
D_EMB = 128
D_ENC = 256
D_DEC = 512
N_CORES = 8
KE = D_ENC // 128      # 2
ME = 4 * D_ENC // 128  # 8
KD = D_DEC // 128      # 4
MD = 4 * D_DEC // 128  # 16
GWE = KE * 32          # 64  (one gate group width, encoder)
GWD = KD * 32          # 128 (one gate group width, decoder)


FP8 = False


class _Stop(Exception):
    pass


def build(S=128, T=128, V=32000, VS=4000, phases=9,
          TC=32,      # attention/FFN/vocab t-chunk
          XCHE=16,    # encoder xproj ring chunk (steps)
          XCHD=8,     # decoder xproj ring chunk (steps)
          PE_BUDGET=6000, NBG_=2, FP8_GATES=None,
          ):
    NI_E = B * S
    NI_D = B * T
    NTC = T // TC
    NVC = VS // 500
    BTC = B * TC              # columns per t-chunk (b-major: b*TC+ti)
    SCALE = 1.0 / float(np.sqrt(np.float32(2 * D_ENC)))

    nc = bacc.Bacc("TRN2", target_bir_lowering=False, debug=False)

    # ---- external inputs ----
    tokb = nc.dram_tensor("tokb", [V, D_EMB], BF16, kind="ExternalInput")
    idx_e = nc.dram_tensor("idx_e", [128, NI_E // 16], I16, kind="ExternalInput")
    idx_d = nc.dram_tensor("idx_d", [128, NI_D // 16], I16, kind="ExternalInput")
    startT = nc.dram_tensor("startT", [128, 1], BF16, kind="ExternalInput")
    diff_e = nc.dram_tensor("diff_e", [1, 2 * D_ENC], BF16, kind="ExternalInput")
    e0T = nc.dram_tensor("e0T", [128, KD], F32, kind="ExternalInput")
    lab_i = nc.dram_tensor("lab_i", [1, B], BF16, kind="ExternalInput")
    diff_s = nc.dram_tensor("diff_s", [1, D_DEC], BF16, kind="ExternalInput")
    s0T = nc.dram_tensor("s0T", [128, KD], F32, kind="ExternalInput")
    lab_d = nc.dram_tensor("lab_d", [1, B], BF16, kind="ExternalInput")
    wih_f = nc.dram_tensor("wih_f", [128, ME * 128], BF16, kind="ExternalInput")
    wih_b = nc.dram_tensor("wih_b", [128, ME * 128], BF16, kind="ExternalInput")
    wih_d = nc.dram_tensor("wih_d", [128, MD * 128], BF16, kind="ExternalInput")
    if FP8_GATES is None:
        FP8_GATES = FP8
    WDT = mybir.dt.float8e4 if FP8_GATES else BF16
    GSC = 1.0 / 64.0 if FP8_GATES else 1.0
    XSC = 64.0 if FP8_GATES else 1.0
    whh_f = nc.dram_tensor("whh_f", [128, KE * ME * 128], WDT, kind="ExternalInput")
    whh_b = nc.dram_tensor("whh_b", [128, KE * ME * 128], WDT, kind="ExternalInput")
    whh_d = nc.dram_tensor("whh_d", [128, KD * MD * 128], WDT, kind="ExternalInput")
    wtr = nc.dram_tensor("wtr", [128, KD * KD * 128], BF16, kind="ExternalInput")
    wf1 = nc.dram_tensor("wf1", [128, 8 * KD * 128], BF16, kind="ExternalInput")
    wf2 = nc.dram_tensor("wf2", [128, KD * VS], BF16, kind="ExternalInput")
    bs_f = nc.dram_tensor("bs_f", [128, ME], F32, kind="ExternalInput")
    bs_b = nc.dram_tensor("bs_b", [128, ME], F32, kind="ExternalInput")
    bs_d = nc.dram_tensor("bs_d", [128, MD], F32, kind="ExternalInput")
    b1a = nc.dram_tensor("b1a", [128, KD], F32, kind="ExternalInput")
    b1h = nc.dram_tensor("b1h", [128, KD], F32, kind="ExternalInput")

    out = nc.dram_tensor("out", [B, T, VS], BF16, kind="ExternalOutput")
    dbg_mem = nc.dram_tensor("dbg_mem", [128, 2 * KE * B * S], BF16,
                             kind="ExternalOutput")
    dbg_cd = nc.dram_tensor("dbg_cd", [128, KD * B], F32, kind="ExternalOutput")
    dbg_h = nc.dram_tensor("dbg_h", [128, KD * B * T], BF16, kind="ExternalOutput")
    wf2_3d = wf2.ap().rearrange("p (k v) -> p k v", k=KD)

    with tile.TileContext(nc) as tc, ExitStack() as ctx:
        # ---------------- persistent pools ----------------
        wpool = ctx.enter_context(tc.tile_pool(name="weights", bufs=1))
        spool = ctx.enter_context(tc.tile_pool(name="state", bufs=1))
        big = ctx.enter_context(tc.tile_pool(name="big", bufs=1))

        def load(dram, shape, dtype, tag, pool=None):
            t = (pool or wpool).tile(shape, dtype, tag=tag, name=tag)
            nc.sync.dma_start(t[:], dram[:, :])
            return t

        wih_d_s = load(wih_d, [128, MD * 128], BF16, "wih_d")
        bs_d_s = load(bs_d, [128, MD], F32, "bs_d")
        b1a_s = load(b1a, [128, KD], F32, "b1a")
        b1h_s = load(b1h, [128, KD], F32, "b1h")
        startT_s = load(startT, [128, 1], BF16, "startT")
        e0T_s = load(e0T, [128, KD], F32, "e0T")
        s0T_s = load(s0T, [128, KD], F32, "s0T")
        ident = wpool.tile([128, 128], BF16, tag="ident", name="ident")
        make_identity(nc, ident)

        diff_e_s = wpool.tile([1, 2 * D_ENC], BF16, tag="diff_e", name="diff_e")
        nc.sync.dma_start(diff_e_s[:], diff_e[:, :])
        diff_s_s = wpool.tile([1, D_DEC], BF16, tag="diff_s", name="diff_s")
        nc.sync.dma_start(diff_s_s[:], diff_s[:, :])
        lab_i_s = wpool.tile([1, B], BF16, tag="lab_i", name="lab_i")
        nc.sync.dma_start(lab_i_s[:], lab_i[:, :])
        lab_d_s = wpool.tile([1, B], BF16, tag="lab_d", name="lab_d")
        nc.sync.dma_start(lab_d_s[:], lab_d[:, :])

        whh_d_s = load(whh_d, [128, KD * MD * 128], WDT, "whh_d")
        wf1_s = load(wf1, [128, 8 * KD * 128], BF16, "wf1")

        # ---------------- gathers ----------------
        idx_e_s = wpool.tile([128, NI_E // 16], I16, tag="idx_e", name="idx_e")
        nc.sync.dma_start(idx_e_s[:], idx_e[:, :])
        idx_d_s = wpool.tile([128, NI_D // 16], I16, tag="idx_d", name="idx_d")
        nc.sync.dma_start(idx_d_s[:], idx_d[:, :])
        decT = big.tile([128, 1, NI_D], BF16, tag="decT", name="decT")  # cols t*32+b
        nc.gpsimd.dma_gather(decT[:], tokb[:, :], idx_d_s[:], NI_D, NI_D,
                             D_EMB, transpose=True, single_packet=False)
        nc.vector.tensor_copy(decT[:, 0, 0:B],
                              startT_s[:, 0:1].to_broadcast((128, B)))

        # ---------------- big state tensors ----------------
        mem_T = big.tile([128, 2 * KE, B, S], BF16, tag="mem_T", name="mem_T")
        memT4 = mem_T[:]
        memN = big.tile([128, B, 2 * KE * 128], BF16, tag="memN", name="memN")

        # ---------------- init h0 / c ----------------
        h0f = spool.tile([128, KE * B], BF16, tag="h0f", name="h0f")
        h0b = spool.tile([128, KE * B], BF16, tag="h0b", name="h0b")
        h0d = spool.tile([128, KD * B], BF16, tag="h0d", name="h0d")
        c_f = spool.tile([128, KE * B], F32, tag="c_f", name="c_f")
        c_b = spool.tile([128, KE * B], F32, tag="c_b", name="c_b")
        c_d = spool.tile([128, KD * B], F32, tag="c_d", name="c_d")
        nc.vector.memset(c_f[:], 0.0)
        nc.vector.memset(c_b[:], 0.0)

        with tc.tile_pool(name="init_ps", bufs=2, space="PSUM") as ips, \
             tc.tile_pool(name="init_sb", bufs=2) as isb:
            for dst, dbase in ((h0f, 0), (h0b, KE)):
                for k in range(KE):
                    ps = ips.tile([128, B], F32, tag="i", name="i")
                    col = (dbase + k) * 128
                    nc.tensor.matmul(ps[:], diff_e_s[:, col:col + 128],
                                     lab_i_s[:, :], start=True, stop=True)
                    f32t = isb.tile([128, B], F32, tag="h0t", name="h0t")
                    nc.vector.tensor_scalar_add(f32t[:], ps[:],
                                                e0T_s[:, dbase + k:dbase + k + 1])
                    nc.vector.tensor_copy(dst[:, k * B:(k + 1) * B], f32t[:])
            for k in range(KD):
                ps = ips.tile([128, B], F32, tag="i", name="i")
                nc.tensor.matmul(ps[:], diff_s_s[:, k * 128:(k + 1) * 128],
                                 lab_d_s[:, :], start=True, stop=True)
                f32t = isb.tile([128, B], F32, tag="h0t", name="h0t")
                nc.vector.tensor_scalar_add(f32t[:], ps[:], s0T_s[:, k:k + 1])
                nc.vector.tensor_copy(h0d[:, k * B:(k + 1) * B], f32t[:])

        try:
            if phases < 2:
                raise _Stop

            # ================= ENCODER =================
            NCH_E = S // XCHE
            with tc.tile_pool(name="encw", bufs=1) as encw, \
                 tc.tile_pool(name="xe_ps", bufs=2, space="PSUM") as xeps, \
                 tc.tile_pool(name="xr_f", bufs=3) as xrf, \
                 tc.tile_pool(name="xr_b", bufs=3) as xrb, \
                 tc.tile_pool(name="eg_ps", bufs=2, space="PSUM") as egps, \
                 tc.tile_pool(name="eg_sb", bufs=2) as egsb:

                wih_f_s = load(wih_f, [128, ME * 128], BF16, "wih_f", pool=encw)
                wih_b_s = load(wih_b, [128, ME * 128], BF16, "wih_b", pool=encw)
                whh_f_s = load(whh_f, [128, KE * ME * 128], WDT, "whh_f", pool=encw)
                whh_b_s = load(whh_b, [128, KE * ME * 128], WDT, "whh_b", pool=encw)
                wtr_s = load(wtr, [128, KD * KD * 128], BF16, "wtr", pool=encw)
                bs_f_s = load(bs_f, [128, ME], F32, "bs_f", pool=encw)
                bs_b_s = load(bs_b, [128, ME], F32, "bs_b", pool=encw)
                encT = encw.tile([128, 1, NI_E], BF16, tag="encT", name="encT")
                nc.gpsimd.dma_gather(encT[:], tokb[:, :], idx_e_s[:], NI_E, NI_E,
                                     D_EMB, transpose=True, single_packet=False)

                xch_f = {}
                xch_b = {}

                def emit_xproj_e(d, c):
                    pool, store, w_s, bias = ((xrf, xch_f, wih_f_s, bs_f_s) if d == 0
                                              else (xrb, xch_b, wih_b_s, bs_b_s))
                    t = pool.tile([128, XCHE, ME * 32], BF16, tag=f"x{d}",
                                  name=f"x{d}")
                    store[c] = t
                    s0 = c * XCHE
                    units = []
                    for m in range(ME):
                        def unit(m=m, t=t, s0=s0, w_s=w_s, bias=bias):
                            ps = xeps.tile([128, XCHE * 32], F32, tag="xe", name="xe")
                            nc.tensor.matmul(ps[:], w_s[:, m * 128:(m + 1) * 128],
                                             encT[:, 0, s0 * 32:(s0 + XCHE) * 32],
                                             start=True, stop=True)
                            dst = t[:, :, m * 32:(m + 1) * 32]
                            srcv = ps[:].rearrange("p (s b) -> p s b", b=32)
                            if m % 2 == 0:
                                nc.scalar.activation(dst, srcv, AF.Identity,
                                                     scale=XSC,
                                                     bias=bias[:, m:m + 1])
                            else:
                                nc.vector.tensor_scalar(dst, srcv, XSC,
                                                        bias[:, m:m + 1],
                                                        ALU.mult, ALU.add)
                        units.append(unit)
                    return units

                q_enc = []
                for cwin in (0, 1):
                    for u in emit_xproj_e(0, cwin):
                        u()
                    for u in emit_xproj_e(1, NCH_E - 1 - cwin):
                        u()
                next_win = 2

                morder_e = (list(range(KE, 2 * KE)) + list(range(2 * KE, 3 * KE))
                            + list(range(0, KE)) + list(range(3 * KE, 4 * KE)))

                h_prev = None
                h0e = egsb.tile([128, 2 * KE * B], BF16, tag="h0e", name="h0e")
                nc.vector.tensor_copy(h0e[:, 0:KE * B], h0f[:])
                nc.vector.tensor_copy(h0e[:, KE * B:2 * KE * B], h0b[:])
                c_e = spool.tile([128, 2 * KE * B], F32, tag="c_e", name="c_e")
                nc.vector.memset(c_e[:], 0.0)
                HW_E = KE * B  # 64: one direction's h width
                for step in range(S):
                    if step % XCHE == 0 and next_win < NCH_E:
                        q_enc.extend(emit_xproj_e(0, next_win))
                        q_enc.extend(emit_xproj_e(1, NCH_E - 1 - next_win))
                        next_win += 1
                    for _ in range(2):
                        if q_enc:
                            q_enc.pop(0)()

                    s_f = step
                    s_b = S - 1 - step
                    xt_f = xch_f[s_f // XCHE]
                    xt_b = xch_b[s_b // XCHE]
                    hc = h0e if step == 0 else h_prev

                    ps = egps.tile([128, 2 * ME * 32], F32, tag="g", name="g")
                    ps4 = ps[:].rearrange("p (d g c) -> p d g c", d=2, g=4)
                    nc.tensor.matmul(ps[:, 0:ME * 32], ident[:, :],
                                     xt_f[:, s_f % XCHE, :],
                                     start=True, stop=False, skip_group_check=True)
                    nc.tensor.matmul(ps[:, ME * 32:2 * ME * 32], ident[:, :],
                                     xt_b[:, s_b % XCHE, :],
                                     start=False, stop=False, skip_group_check=True)
                    n_mm = 2 * len(morder_e) * KE
                    i_mm = 0
                    for m in morder_e:
                        for d in (0, 1):
                            whh_s = whh_f_s if d == 0 else whh_b_s
                            off = d * ME * 32 + m * 32
                            for k in range(KE):
                                rhs = hc[:, d * HW_E + k * B:d * HW_E + (k + 1) * B]
                                lt = whh_s[:, (k * ME + m) * 128:
                                           (k * ME + m + 1) * 128]
                                i_mm += 1
                                nc.tensor.matmul(
                                    ps[:, off:off + 32], lt, rhs,
                                    start=False, stop=(i_mm == n_mm),
                                    skip_group_check=True)
                    gf = egsb.tile([128, 2, GWE], F32, tag="gf", name="gf")
                    gg = egsb.tile([128, 2, GWE], F32, tag="gg", name="gg")
                    gi = egsb.tile([128, 2, GWE], F32, tag="gi", name="gi")
                    go = egsb.tile([128, 2, GWE], F32, tag="go", name="go")
                    t1 = egsb.tile([128, 2 * GWE], F32, tag="t1", name="t1")
                    t2 = egsb.tile([128, 2 * GWE], F32, tag="t2", name="t2")
                    tct = egsb.tile([128, 2 * GWE], F32, tag="tct", name="tct")
                    nc.scalar.activation(gf[:], ps4[:, :, 1, :], AF.Sigmoid,
                                         scale=GSC)
                    nc.scalar.activation(gg[:], ps4[:, :, 2, :], AF.Tanh,
                                         scale=GSC)
                    nc.scalar.activation(gi[:], ps4[:, :, 0, :], AF.Sigmoid,
                                         scale=GSC)
                    nc.scalar.activation(go[:], ps4[:, :, 3, :], AF.Sigmoid,
                                         scale=GSC)
                    gff = gf[:].rearrange("p d c -> p (d c)")
                    ggf = gg[:].rearrange("p d c -> p (d c)")
                    gif = gi[:].rearrange("p d c -> p (d c)")
                    gof = go[:].rearrange("p d c -> p (d c)")
                    nc.vector.tensor_tensor(t1[:], gff, c_e[:], ALU.mult)
                    nc.vector.tensor_tensor(t2[:], gif, ggf, ALU.mult)
                    nc.vector.tensor_tensor(c_e[:], t1[:], t2[:], ALU.add)
                    nc.scalar.activation(tct[:], c_e[:], AF.Tanh)
                    hn = egsb.tile([128, 2 * KE * B], BF16, tag="hn", name="hn")
                    nc.vector.tensor_tensor(hn[:], gof, tct[:], ALU.mult)
                    h_prev = hn
                    nc.vector.tensor_copy(
                        memT4[:, 0:KE, :, s_f],
                        hn[:, 0:HW_E].rearrange("p (k b) -> p k b", k=KE))
                    nc.vector.tensor_copy(
                        memT4[:, KE:2 * KE, :, s_b],
                        hn[:, HW_E:2 * HW_E].rearrange("p (k b) -> p k b", k=KE))

                while q_enc:
                    q_enc.pop(0)()

                if phases < 3:
                    raise _Stop

                # ---------- c_t (decoder c init) ----------
                ccT = egsb.tile([128, KD * B], BF16, tag="ccT", name="ccT")
                nc.vector.tensor_copy(ccT[:], c_e[:])
                with tc.tile_pool(name="ct_ps", bufs=2, space="PSUM") as cps, \
                     tc.tile_pool(name="ct_sb", bufs=2) as csb:
                    for m in range(KD):
                        ps = cps.tile([128, B], F32, tag="ct", name="ct")
                        for k in range(KD):
                            lt = wtr_s[:, (k * KD + m) * 128:(k * KD + m + 1) * 128]
                            nc.tensor.matmul(ps[:], lt, ccT[:, k * B:(k + 1) * B],
                                             start=(k == 0), stop=(k == KD - 1))
                        ab = csb.tile([128, B], F32, tag="ab", name="ab")
                        nc.scalar.activation(ab[:], ps[:], AF.Abs)
                        idt = csb.tile([128, B], F32, tag="idt", name="idt")
                        nc.scalar.activation(idt[:], ps[:], AF.Identity, scale=0.55)
                        nc.vector.scalar_tensor_tensor(
                            c_d[:, m * B:(m + 1) * B], ab[:],
                            0.45, idt[:], ALU.mult, ALU.add)

            if phases < 4:
                raise _Stop

            # ================= DECODER =================
            NCH_D = T // XCHD
            with tc.tile_pool(name="xd_ps", bufs=1, space="PSUM") as xdps, \
                 tc.tile_pool(name="xr_d", bufs=3) as xrd, \
                 tc.tile_pool(name="dg_ps", bufs=2, space="PSUM") as dgps, \
                 tc.tile_pool(name="dg_sb", bufs=2) as dgsb, \
                 tc.tile_pool(name="h_ring", bufs=3) as hpool, \
                 tc.tile_pool(name="at_ps", bufs=2, space="PSUM") as aps, \
                 tc.tile_pool(name="at_sb", bufs=2) as asb, \
                 tc.tile_pool(name="md_ps", bufs=1, space="PSUM") as mps, \
                 tc.tile_pool(name="md_sb", bufs=2) as msb, \
                 tc.tile_pool(name="cx_sb", bufs=1) as cxsb, \
                 tc.tile_pool(name="lg_ps", bufs=2, space="PSUM") as lps, \
                 tc.tile_pool(name="lg_sb", bufs=2) as lsb, \
                 tc.tile_pool(name="wf2_sb", bufs=2) as wfp:

                xch_d = {}

                def emit_xproj_d(c):
                    t = xrd.tile([128, XCHD, MD * 32], BF16, tag="xd", name="xd")
                    xch_d[c] = t
                    t0 = c * XCHD
                    units = []
                    for m in range(MD):
                        def unit(m=m, t=t, t0=t0):
                            ps = xdps.tile([128, XCHD * 32], F32, tag="xd",
                                           name="xd")
                            nc.tensor.matmul(ps[:],
                                             wih_d_s[:, m * 128:(m + 1) * 128],
                                             decT[:, 0, t0 * 32:(t0 + XCHD) * 32],
                                             start=True, stop=True)
                            dst = t[:, :, m * 32:(m + 1) * 32]
                            srcv = ps[:].rearrange("p (s b) -> p s b", b=32)
                            if m % 2 == 0:
                                nc.scalar.activation(dst, srcv, AF.Identity,
                                                     scale=XSC,
                                                     bias=bs_d_s[:, m:m + 1])
                            else:
                                nc.vector.tensor_scalar(dst, srcv, XSC,
                                                        bs_d_s[:, m:m + 1],
                                                        ALU.mult, ALU.add)
                        units.append(unit)
                    return units

                for cc in (0, 1):
                    for u in emit_xproj_d(cc):
                        u()
                next_xd = 2

                q_hi = []
                q_lo = []  # items: (chunk_tag, closure, pe_cost)

                def drain(n_hi=1, pe_budget=2600, upto_chunk=None):
                    while q_hi and n_hi > 0:
                        q_hi.pop(0)()
                        n_hi -= 1
                    if upto_chunk is not None:
                        while q_lo and q_lo[0][0] <= upto_chunk:
                            q_lo.pop(0)[1]()
                    spent = 0
                    while q_lo and spent < pe_budget:
                        tag, fn, cost = q_lo.pop(0)
                        fn()
                        spent += cost

                def memN_unit(b):
                    def fn():
                        for kd in range(2 * KE):
                            tp = aps.tile([128, 128], BF16, tag="a", name="tp",
                                          padded_shape=[128, 128])
                            nc.tensor.transpose(tp[:], memT4[:, kd, b, :],
                                                ident[:, :])
                            nc.vector.tensor_copy(
                                memN[:, b, kd * 128:(kd + 1) * 128], tp[:])
                    return fn
                for b in range(B):
                    q_lo.append((-1, memN_unit(b), 800))

                Hchunks = {}
                mid_ch = {}
                ctx_ch = {}
                wf2_cur = [None]

                NBG = NBG_  # b's per attention group (stacked psum rows at 0/32)

                def attn_unit(c, g):
                    def fn():
                        Hc = Hchunks[c]
                        if g == 0:
                            ctx_ch[c] = cxsb.tile([128, KD, BTC], BF16,
                                                  tag="ctx", name="ctx")
                        sc = aps.tile([NBG * TC, S], F32, tag="a", name="sc")
                        for j in range(NBG):
                            b = g * NBG + j
                            for k in range(2 * KE):
                                nc.tensor.matmul(sc[j * TC:(j + 1) * TC, :],
                                                 Hc[:, k, b, :],
                                                 memT4[:, k, b, :],
                                                 start=(k == 0),
                                                 stop=(k == 2 * KE - 1))
                        # softmax via sigmoid: e^x = sig(x)/(1-sig(x))
                        NP = NBG * TC
                        sg = asb.tile([NP, S], F32, tag="sg", name="sg")
                        nc.scalar.activation(sg[:], sc[:], AF.Sigmoid, scale=SCALE)
                        # om = 1 - sg on the scalar engine (Identity table)
                        om = asb.tile([NP, S], F32, tag="om", name="om")
                        nc.scalar.activation(om[:], sg[:], AF.Identity,
                                             scale=-1.0, bias=1.0)
                        orc = asb.tile([NP, S], F32, tag="orc", name="orc")
                        nc.vector.reciprocal(orc[:], om[:])
                        e = asb.tile([NP, S], F32, tag="e", name="e")
                        nc.vector.tensor_tensor(e[:], sg[:], orc[:], ALU.mult)
                        den = asb.tile([NP, 1], F32, tag="den", name="den")
                        nc.vector.tensor_reduce(den[:], e[:], AX.X, ALU.add)
                        rec = asb.tile([NP, 1], F32, tag="rec", name="rec")
                        nc.vector.reciprocal(rec[:], den[:])
                        p_sb = asb.tile([NP, S], BF16, tag="p", name="p")
                        nc.vector.tensor_scalar_mul(p_sb[:], e[:], rec[:])
                        for j in range(NBG):
                            b = g * NBG + j
                            pt = aps.tile([S, TC], BF16, tag="a", name="pt",
                                          padded_shape=[128, TC])
                            nc.tensor.transpose(
                                pt[:], p_sb[j * TC:(j + 1) * TC, :],
                                ident[j * TC:(j + 1) * TC, j * TC:(j + 1) * TC])
                            pt_sb = asb.tile([S, TC], BF16, tag="pts", name="pts")
                            nc.scalar.copy(pt_sb[:], pt[:])
                            cx = aps.tile([128, KD * TC], F32, tag="a", name="cx")
                            for kd in range(KD):
                                nc.tensor.matmul(
                                    cx[:, kd * TC:(kd + 1) * TC],
                                    memN[:, b, kd * 128:(kd + 1) * 128],
                                    pt_sb[:], start=True, stop=True)
                            nc.vector.tensor_copy(
                                ctx_ch[c][:, :, b * TC:(b + 1) * TC],
                                cx[:].rearrange("p (k t) -> p k t", k=KD))
                    return fn

                def ffn_unit(c, m, half):
                    def fn():
                        Hc = Hchunks[c]
                        if m == 0 and half == 0:
                            mid_ch[c] = msb.tile([128, KD, BTC], BF16,
                                                 tag="mid", name="mid")
                        HW2 = BTC // 2
                        ps = mps.tile([128, HW2], F32, tag="md", name="md")
                        Hflat = Hc[:].rearrange("p k b t -> p (k b t)")
                        for k in range(KD):
                            lt = wf1_s[:, (k * KD + m) * 128:
                                       (k * KD + m + 1) * 128]
                            nc.tensor.matmul(ps[:], lt,
                                             Hflat[:, k * BTC + half * HW2:
                                                   k * BTC + (half + 1) * HW2],
                                             start=(k == 0), stop=False)
                        ctxf = ctx_ch[c][:].rearrange("p k t -> p (k t)")
                        for k in range(KD):
                            kk = KD + k
                            lt = wf1_s[:, (kk * KD + m) * 128:
                                       (kk * KD + m + 1) * 128]
                            nc.tensor.matmul(ps[:], lt,
                                             ctxf[:, k * BTC + half * HW2:
                                                  k * BTC + (half + 1) * HW2],
                                             start=False, stop=(k == KD - 1))
                        ab = asb.tile([128, HW2], F32, tag="mab", name="mab")
                        nc.scalar.activation(ab[:], ps[:], AF.Abs,
                                             bias=b1a_s[:, m:m + 1])
                        idt = asb.tile([128, HW2], F32, tag="midt", name="midt")
                        nc.scalar.activation(idt[:], ps[:], AF.Identity,
                                             scale=0.55, bias=b1h_s[:, m:m + 1])
                        nc.vector.scalar_tensor_tensor(
                            mid_ch[c][:, m, half * HW2:(half + 1) * HW2],
                            ab[:], 0.45, idt[:], ALU.mult, ALU.add)
                    return fn

                def wf2_unit(vc):
                    def fn():
                        t = wfp.tile([128, KD, 500], BF16, tag="wf2c", name="wf2c")
                        nc.sync.dma_start(t[:],
                                          wf2_3d[:, :, vc * 500:(vc + 1) * 500])
                        wf2_cur[0] = t
                    return fn

                def vocab_unit(c, vc, grp):
                    def fn():
                        wt = wf2_cur[0]
                        midf = mid_ch[c][:].rearrange("p k t -> p (k t)")
                        ps = lps.tile([128, 500], F32, tag="lg", name="lg")
                        for k in range(KD):
                            lt = midf[:, k * BTC + grp * 128:
                                      k * BTC + (grp + 1) * 128]
                            nc.tensor.matmul(ps[:], lt, wt[:, k, :],
                                             start=(k == 0), stop=(k == KD - 1))
                        lo = lsb.tile([128, 500], BF16, tag="lo", name="lo")
                        if grp % 2 == 0:
                            nc.scalar.copy(lo[:], ps[:])
                        else:
                            nc.vector.tensor_copy(lo[:], ps[:])
                        nb = 128 // TC
                        b0 = grp * nb
                        for bl in range(nb):
                            nc.sync.dma_start(
                                out.ap()[b0 + bl, c * TC:(c + 1) * TC,
                                         vc * 500:(vc + 1) * 500],
                                lo[bl * TC:(bl + 1) * TC, :])
                    return fn

                def queue_chunk_work(c):
                    for g in range(B // NBG):
                        q_lo.append((c, attn_unit(c, g), 800))
                    for m in range(KD):
                        for half in range(2):
                            q_lo.append((c, ffn_unit(c, m, half), 1750))
                    for vc in range(NVC):
                        q_lo.append((c, wf2_unit(vc), 0))
                        for grp in range(BTC // 128):
                            q_lo.append((c, vocab_unit(c, vc, grp), 900))

                morder_d = (list(range(KD, 2 * KD)) + list(range(2 * KD, 3 * KD))
                            + list(range(0, KD)) + list(range(3 * KD, 4 * KD)))

                for t in range(T):
                    tci = t // TC
                    t_loc = t % TC
                    if t_loc == 0:
                        if tci >= 3:
                            drain(n_hi=0, pe_budget=0, upto_chunk=tci - 3)
                        Hchunks[tci] = hpool.tile([128, KD, B, TC], BF16,
                                                  tag="H", name="H")
                    if t % XCHD == 0 and next_xd < NCH_D:
                        q_hi.extend(emit_xproj_d(next_xd))
                        next_xd += 1

                    if phases >= 6:
                        drain(n_hi=2, pe_budget=PE_BUDGET)
                    else:
                        drain(n_hi=2, pe_budget=0)

                    xt = xch_d[t // XCHD]
                    ps = dgps.tile([128, MD * 32], F32, tag="gd", name="gd")
                    nc.tensor.matmul(ps[:], ident[:, :], xt[:, t % XCHD, :],
                                     start=True, stop=False, skip_group_check=True)
                    for mi, m in enumerate(morder_d):
                        for k in range(KD):
                            if t == 0:
                                rhs = h0d[:, k * B:(k + 1) * B]
                            else:
                                rhs = h_prev_d[:, k * B:(k + 1) * B]
                            lt = whh_d_s[:, (k * MD + m) * 128:
                                         (k * MD + m + 1) * 128]
                            nc.tensor.matmul(
                                ps[:, m * 32:(m + 1) * 32], lt, rhs,
                                start=False,
                                stop=(mi == len(morder_d) - 1 and k == KD - 1),
                                skip_group_check=True)
                    gfd = dgsb.tile([128, GWD], F32, tag="gf", name="gf")
                    ggd = dgsb.tile([128, GWD], F32, tag="gg", name="gg")
                    gid = dgsb.tile([128, GWD], F32, tag="gi", name="gi")
                    god = dgsb.tile([128, GWD], F32, tag="go", name="go")
                    t1d = dgsb.tile([128, GWD], F32, tag="t1", name="t1")
                    t2d = dgsb.tile([128, GWD], F32, tag="t2", name="t2")
                    tctd = dgsb.tile([128, GWD], F32, tag="tct", name="tct")
                    nc.scalar.activation(gfd[:], ps[:, GWD:2 * GWD], AF.Sigmoid,
                                         scale=GSC)
                    nc.scalar.activation(ggd[:], ps[:, 2 * GWD:3 * GWD], AF.Tanh,
                                         scale=GSC)
                    nc.scalar.activation(gid[:], ps[:, 0:GWD], AF.Sigmoid,
                                         scale=GSC)
                    nc.scalar.activation(god[:], ps[:, 3 * GWD:4 * GWD], AF.Sigmoid,
                                         scale=GSC)
                    nc.vector.tensor_tensor(t1d[:], gfd[:], c_d[:], ALU.mult)
                    nc.vector.tensor_tensor(t2d[:], gid[:], ggd[:], ALU.mult)
                    nc.vector.tensor_tensor(c_d[:], t1d[:], t2d[:], ALU.add)
                    nc.scalar.activation(tctd[:], c_d[:], AF.Tanh)
                    hn = dgsb.tile([128, KD * B], BF16, tag="hn", name="hn")
                    nc.vector.tensor_tensor(hn[:], god[:], tctd[:], ALU.mult)
                    h_prev_d = hn
                    nc.vector.tensor_copy(
                        Hchunks[tci][:, :, :, t_loc],
                        hn[:].rearrange("p (k b) -> p k b", k=KD))

                    if phases >= 5 and t_loc == TC - 1:
                        queue_chunk_work(tci)

                if phases >= 5:
                    while q_hi:
                        q_hi.pop(0)()
                    while q_lo:
                        q_lo.pop(0)[1]()
        except _Stop:
            pass

    nc.compile()
    return nc


def prep_inputs(i, S=128, T=128, V=32000, VS=4000):
    """Host-side input staging -> list of 8 per-core in_maps."""
    def as_np(x, dt=np.float32):
        return np.ascontiguousarray(np.asarray(x), dtype=dt)

    tok = as_np(i["tok_emb"]).astype(bf16)

    def idx_prep(flat):
        a = flat.astype(np.int16).reshape(-1, 16).T
        return np.ascontiguousarray(np.tile(a, (8, 1)))

    inp = as_np(i["inp"], np.int64)
    x = as_np(i["x"], np.int64)
    idx_e = idx_prep(inp.T.reshape(-1))              # s-major: s*32+b
    dmat = np.zeros((B, T), np.int64)
    dmat[:, 1:] = x[:, :T - 1]
    idx_d = idx_prep(dmat.T.reshape(-1))             # t-major: t*32+b

    startT = as_np(i["start_emb"]).reshape(D_EMB, 1).astype(bf16)

    est = as_np(i["enc_style_emb"])                  # [2, 512]
    diff_e = (est[1] - est[0]).reshape(1, -1).astype(bf16)
    e0T = np.ascontiguousarray(est[0].reshape(KD, 128).T)
    sty = as_np(i["style_emb"])                      # [2, 512]
    diff_s = (sty[1] - sty[0]).reshape(1, -1).astype(bf16)
    s0T = np.ascontiguousarray(sty[0].reshape(KD, 128).T)
    lab_i = as_np(i["label_i"], np.float32).reshape(1, B).astype(bf16)
    lab_d = as_np(i["label"], np.float32).reshape(1, B).astype(bf16)

    def wihT(w, nm):
        a = w.reshape(nm, 128, 128)
        return np.ascontiguousarray(a.transpose(2, 0, 1).reshape(128, nm * 128)
                                    ).astype(bf16)

    def whhT(w, nk, nm):
        a = w.reshape(nm, 128, nk, 128)
        a = a.transpose(3, 2, 0, 1)
        return np.ascontiguousarray(a.reshape(128, nk * nm * 128)).astype(bf16)

    wih_f = wihT(as_np(i["Wih_f"]), ME)
    wih_b = wihT(as_np(i["Wih_b"]), ME)
    wih_d = wihT(as_np(i["Wih_d"]), MD)
    if FP8:
        fp8 = ml_dtypes.float8_e4m3fn
        whh_f = (whhT(as_np(i["Whh_f"]), KE, ME).astype(np.float32) * 64).astype(fp8)
        whh_b = (whhT(as_np(i["Whh_b"]), KE, ME).astype(np.float32) * 64).astype(fp8)
        whh_d = (whhT(as_np(i["Whh_d"]), KD, MD).astype(np.float32) * 64).astype(fp8)
    else:
        whh_f = whhT(as_np(i["Whh_f"]), KE, ME)
        whh_b = whhT(as_np(i["Whh_b"]), KE, ME)
        whh_d = whhT(as_np(i["Whh_d"]), KD, MD)
    wtr = whhT(as_np(i["W_tr"]), KD, KD)
    wf1 = whhT(as_np(i["W_f1"]), 8, KD)

    wf2_full = as_np(i["W_f2"])                      # [V, 512]

    def bcol(v, nm):
        return np.ascontiguousarray(v.reshape(nm, 128).T)

    _xs = 64.0 if FP8 else 1.0
    bs_f = bcol(as_np(i["bih_f"]) + as_np(i["bhh_f"]), ME) * _xs
    bs_b = bcol(as_np(i["bih_b"]) + as_np(i["bhh_b"]), ME) * _xs
    bs_d = bcol(as_np(i["bih_d"]) + as_np(i["bhh_d"]), MD) * _xs
    b1 = as_np(i["b_f1"])
    b1a = bcol(b1, KD)
    b1h = bcol(0.55 * b1, KD)

    common = dict(tokb=tok, idx_e=idx_e, idx_d=idx_d, startT=startT,
                  diff_e=diff_e, e0T=e0T, lab_i=lab_i,
                  diff_s=diff_s, s0T=s0T, lab_d=lab_d,
                  wih_f=wih_f, wih_b=wih_b, wih_d=wih_d,
                  whh_f=whh_f, whh_b=whh_b, whh_d=whh_d,
                  wtr=wtr, wf1=wf1,
                  bs_f=bs_f, bs_b=bs_b, bs_d=bs_d, b1a=b1a, b1h=b1h)
    in_maps = []
    for c in range(N_CORES):
        shard = wf2_full[c * VS:(c + 1) * VS]        # [VS, 512]
        a = shard.reshape(VS, KD, 128)
        wf2c = np.ascontiguousarray(a.transpose(2, 1, 0).reshape(128, KD * VS)
                                    ).astype(bf16)
        in_maps.append(dict(common, wf2=wf2c))
    return in_maps


_NC_CACHE = {}


def kernel(**inputs):
    key = "full"
    if key not in _NC_CACHE:
        _NC_CACHE[key] = build()
    nc = _NC_CACHE[key]
    in_maps = prep_inputs(inputs)
    res = run_bass_kernel_spmd(nc, in_maps, core_ids=list(range(N_CORES)))
    full = np.concatenate([r["out"] for r in res.results], axis=2)
    return full.astype(np.float32)


# revision 22
# speedup vs baseline: 1.1773x; 1.1773x over previous
"""DenoiseLSTM Trainium2 kernel (8 NeuronCores, SPMD), v2.

Structure (single fused program per core; recurrences replicated, vocab
projection sharded over V=32000 -> 4000 cols/core):

  A. weight loads + token gathers + h0 inits.
  B. encoder (fwd+bwd interleaved). Input projections are computed into
     SBUF ring chunks (no DRAM roundtrip) and PRELOADED into the gate
     PSUM via an identity matmul, so the recurrent matmuls accumulate on
     top (start=False) and no vector add sits on the critical path. The
     final gate multiply writes h straight into mem_T (no copy); the
     next step's matmul reads it back strided.
  C. decoder, same gate scheme, h written into an H ring chunk.
     Attention + FFN + vocab projection for each completed 32-step
     t-chunk are emitted into the gaps between decoder steps via a work
     queue, so the big parallel matmuls fill the PE idle time of the
     serial recurrence. Logits are written bf16; the host upcasts.
"""
import sys

sys.path.insert(0, "/opt/trn_rl_repo")

from contextlib import ExitStack

import numpy as np
import ml_dtypes

import concourse.bass as bass
import concourse.bacc as bacc
import concourse.mybir as mybir
import concourse.tile as tile
from concourse.bass_utils import run_bass_kernel_spmd
from concourse.masks import make_identity

bf16 = ml_dtypes.bfloat16
F32 = mybir.dt.float32
BF16 = mybir.dt.bfloat16
I16 = mybir.dt.int16
AF = mybir.ActivationFunctionType
ALU = mybir.AluOpType
AX = mybir.AxisListType

B = 32
D_EMB = 128
D_ENC = 256
D_DEC = 512
N_CORES = 8
KE = D_ENC // 128      # 2
ME = 4 * D_ENC // 128  # 8
KD = D_DEC // 128      # 4
MD = 4 * D_DEC // 128  # 16
GWE = KE * 32          # 64  (one gate group width, encoder)
GWD = KD * 32          # 128 (one gate group width, decoder)


FP8 = False


class _Stop(Exception):
    pass


def build(S=128, T=128, V=32000, VS=4000, phases=9,
          TC=32,      # attention/FFN/vocab t-chunk
          XCHE=16,    # encoder xproj ring chunk (steps)
          XCHD=8,     # decoder xproj ring chunk (steps)
          PE_BUDGET=3200, NBG_=2, FP8_GATES=None,
          ):
    NI_E = B * S
    NI_D = B * T
    NTC = T // TC
    NVC = VS // 500
    BTC = B * TC              # columns per t-chunk (b-major: b*TC+ti)
    SCALE = 1.0 / float(np.sqrt(np.float32(2 * D_ENC)))

    nc = bacc.Bacc("TRN2", target_bir_lowering=False, debug=False)

    # ---- external inputs ----
    tokb = nc.dram_tensor("tokb", [V, D_EMB], BF16, kind="ExternalInput")
    idx_e = nc.dram_tensor("idx_e", [128, NI_E // 16], I16, kind="ExternalInput")
    idx_d = nc.dram_tensor("idx_d", [128, NI_D // 16], I16, kind="ExternalInput")
    startT = nc.dram_tensor("startT", [128, 1], BF16, kind="ExternalInput")
    diff_e = nc.dram_tensor("diff_e", [1, 2 * D_ENC], BF16, kind="ExternalInput")
    e0T = nc.dram_tensor("e0T", [128, KD], F32, kind="ExternalInput")
    lab_i = nc.dram_tensor("lab_i", [1, B], BF16, kind="ExternalInput")
    diff_s = nc.dram_tensor("diff_s", [1, D_DEC], BF16, kind="ExternalInput")
    s0T = nc.dram_tensor("s0T", [128, KD], F32, kind="ExternalInput")
    lab_d = nc.dram_tensor("lab_d", [1, B], BF16, kind="ExternalInput")
    wih_f = nc.dram_tensor("wih_f", [128, ME * 128], BF16, kind="ExternalInput")
    wih_b = nc.dram_tensor("wih_b", [128, ME * 128], BF16, kind="ExternalInput")
    wih_d = nc.dram_tensor("wih_d", [128, MD * 128], BF16, kind="ExternalInput")
    if FP8_GATES is None:
        FP8_GATES = FP8
    WDT = mybir.dt.float8e4 if FP8_GATES else BF16
    GSC = 1.0 / 64.0 if FP8_GATES else 1.0
    XSC = 64.0 if FP8_GATES else 1.0
    whh_f = nc.dram_tensor("whh_f", [128, KE * ME * 128], WDT, kind="ExternalInput")
    whh_b = nc.dram_tensor("whh_b", [128, KE * ME * 128], WDT, kind="ExternalInput")
    whh_d = nc.dram_tensor("whh_d", [128, KD * MD * 128], WDT, kind="ExternalInput")
    wtr = nc.dram_tensor("wtr", [128, KD * KD * 128], BF16, kind="ExternalInput")
    wf1 = nc.dram_tensor("wf1", [128, 8 * KD * 128], BF16, kind="ExternalInput")
    wf2 = nc.dram_tensor("wf2", [128, KD * VS], BF16, kind="ExternalInput")
    bs_f = nc.dram_tensor("bs_f", [128, ME], F32, kind="ExternalInput")
    bs_b = nc.dram_tensor("bs_b", [128, ME], F32, kind="ExternalInput")
    bs_d = nc.dram_tensor("bs_d", [128, MD], F32, kind="ExternalInput")
    b1a = nc.dram_tensor("b1a", [128, KD], F32, kind="ExternalInput")
    b1h = nc.dram_tensor("b1h", [128, KD], F32, kind="ExternalInput")

    out = nc.dram_tensor("out", [B, T, VS], BF16, kind="ExternalOutput")
    dbg_mem = nc.dram_tensor("dbg_mem", [128, 2 * KE * B * S], BF16,
                             kind="ExternalOutput")
    dbg_cd = nc.dram_tensor("dbg_cd", [128, KD * B], F32, kind="ExternalOutput")
    dbg_h = nc.dram_tensor("dbg_h", [128, KD * B * T], BF16, kind="ExternalOutput")
    wf2_3d = wf2.ap().rearrange("p (k v) -> p k v", k=KD)

    with tile.TileContext(nc) as tc, ExitStack() as ctx:
        # ---------------- persistent pools ----------------
        wpool = ctx.enter_context(tc.tile_pool(name="weights", bufs=1))
        spool = ctx.enter_context(tc.tile_pool(name="state", bufs=1))
        big = ctx.enter_context(tc.tile_pool(name="big", bufs=1))

        def load(dram, shape, dtype, tag, pool=None):
            t = (pool or wpool).tile(shape, dtype, tag=tag, name=tag)
            nc.sync.dma_start(t[:], dram[:, :])
            return t

        wih_d_s = load(wih_d, [128, MD * 128], BF16, "wih_d")
        bs_d_s = load(bs_d, [128, MD], F32, "bs_d")
        b1a_s = load(b1a, [128, KD], F32, "b1a")
        b1h_s = load(b1h, [128, KD], F32, "b1h")
        startT_s = load(startT, [128, 1], BF16, "startT")
        e0T_s = load(e0T, [128, KD], F32, "e0T")
        s0T_s = load(s0T, [128, KD], F32, "s0T")
        ident = wpool.tile([128, 128], BF16, tag="ident", name="ident")
        make_identity(nc, ident)

        diff_e_s = wpool.tile([1, 2 * D_ENC], BF16, tag="diff_e", name="diff_e")
        nc.sync.dma_start(diff_e_s[:], diff_e[:, :])
        diff_s_s = wpool.tile([1, D_DEC], BF16, tag="diff_s", name="diff_s")
        nc.sync.dma_start(diff_s_s[:], diff_s[:, :])
        lab_i_s = wpool.tile([1, B], BF16, tag="lab_i", name="lab_i")
        nc.sync.dma_start(lab_i_s[:], lab_i[:, :])
        lab_d_s = wpool.tile([1, B], BF16, tag="lab_d", name="lab_d")
        nc.sync.dma_start(lab_d_s[:], lab_d[:, :])

        whh_d_s = load(whh_d, [128, KD * MD * 128], WDT, "whh_d")
        wf1_s = load(wf1, [128, 8 * KD * 128], BF16, "wf1")

        # ---------------- gathers ----------------
        idx_e_s = wpool.tile([128, NI_E // 16], I16, tag="idx_e", name="idx_e")
        nc.sync.dma_start(idx_e_s[:], idx_e[:, :])
        idx_d_s = wpool.tile([128, NI_D // 16], I16, tag="idx_d", name="idx_d")
        nc.sync.dma_start(idx_d_s[:], idx_d[:, :])
        decT = big.tile([128, 1, NI_D], BF16, tag="decT", name="decT")  # cols t*32+b
        nc.gpsimd.dma_gather(decT[:], tokb[:, :], idx_d_s[:], NI_D, NI_D,
                             D_EMB, transpose=True, single_packet=False)
        nc.vector.tensor_copy(decT[:, 0, 0:B],
                              startT_s[:, 0:1].to_broadcast((128, B)))

        # ---------------- big state tensors ----------------
        mem_T = big.tile([128, 2 * KE, B, S], BF16, tag="mem_T", name="mem_T")
        memT4 = mem_T[:]
        memN = big.tile([128, B, 2 * KE * 128], BF16, tag="memN", name="memN")

        # ---------------- init h0 / c ----------------
        h0f = spool.tile([128, KE * B], BF16, tag="h0f", name="h0f")
        h0b = spool.tile([128, KE * B], BF16, tag="h0b", name="h0b")
        h0d = spool.tile([128, KD * B], BF16, tag="h0d", name="h0d")
        c_f = spool.tile([128, KE * B], F32, tag="c_f", name="c_f")
        c_b = spool.tile([128, KE * B], F32, tag="c_b", name="c_b")
        c_d = spool.tile([128, KD * B], F32, tag="c_d", name="c_d")
        nc.vector.memset(c_f[:], 0.0)
        nc.vector.memset(c_b[:], 0.0)

        with tc.tile_pool(name="init_ps", bufs=2, space="PSUM") as ips, \
             tc.tile_pool(name="init_sb", bufs=2) as isb:
            for dst, dbase in ((h0f, 0), (h0b, KE)):
                for k in range(KE):
                    ps = ips.tile([128, B], F32, tag="i", name="i")
                    col = (dbase + k) * 128
                    nc.tensor.matmul(ps[:], diff_e_s[:, col:col + 128],
                                     lab_i_s[:, :], start=True, stop=True)
                    f32t = isb.tile([128, B], F32, tag="h0t", name="h0t")
                    nc.vector.tensor_scalar_add(f32t[:], ps[:],
                                                e0T_s[:, dbase + k:dbase + k + 1])
                    nc.vector.tensor_copy(dst[:, k * B:(k + 1) * B], f32t[:])
            for k in range(KD):
                ps = ips.tile([128, B], F32, tag="i", name="i")
                nc.tensor.matmul(ps[:], diff_s_s[:, k * 128:(k + 1) * 128],
                                 lab_d_s[:, :], start=True, stop=True)
                f32t = isb.tile([128, B], F32, tag="h0t", name="h0t")
                nc.vector.tensor_scalar_add(f32t[:], ps[:], s0T_s[:, k:k + 1])
                nc.vector.tensor_copy(h0d[:, k * B:(k + 1) * B], f32t[:])

        try:
            if phases < 2:
                raise _Stop

            # ================= ENCODER =================
            NCH_E = S // XCHE
            with tc.tile_pool(name="encw", bufs=1) as encw, \
                 tc.tile_pool(name="xe_ps", bufs=2, space="PSUM") as xeps, \
                 tc.tile_pool(name="xr_f", bufs=3) as xrf, \
                 tc.tile_pool(name="xr_b", bufs=3) as xrb, \
                 tc.tile_pool(name="eg_ps", bufs=2, space="PSUM") as egps, \
                 tc.tile_pool(name="eg_sb", bufs=2) as egsb:

                wih_f_s = load(wih_f, [128, ME * 128], BF16, "wih_f", pool=encw)
                wih_b_s = load(wih_b, [128, ME * 128], BF16, "wih_b", pool=encw)
                whh_f_s = load(whh_f, [128, KE * ME * 128], WDT, "whh_f", pool=encw)
                whh_b_s = load(whh_b, [128, KE * ME * 128], WDT, "whh_b", pool=encw)
                wtr_s = load(wtr, [128, KD * KD * 128], BF16, "wtr", pool=encw)
                bs_f_s = load(bs_f, [128, ME], F32, "bs_f", pool=encw)
                bs_b_s = load(bs_b, [128, ME], F32, "bs_b", pool=encw)
                encT = encw.tile([128, 1, NI_E], BF16, tag="encT", name="encT")
                nc.gpsimd.dma_gather(encT[:], tokb[:, :], idx_e_s[:], NI_E, NI_E,
                                     D_EMB, transpose=True, single_packet=False)

                xch_f = {}
                xch_b = {}

                def emit_xproj_e(d, c):
                    pool, store, w_s, bias = ((xrf, xch_f, wih_f_s, bs_f_s) if d == 0
                                              else (xrb, xch_b, wih_b_s, bs_b_s))
                    t = pool.tile([128, XCHE, ME * 32], BF16, tag=f"x{d}",
                                  name=f"x{d}")
                    store[c] = t
                    s0 = c * XCHE
                    units = []
                    for m in range(ME):
                        def unit(m=m, t=t, s0=s0, w_s=w_s, bias=bias):
                            ps = xeps.tile([128, XCHE * 32], F32, tag="xe", name="xe")
                            nc.tensor.matmul(ps[:], w_s[:, m * 128:(m + 1) * 128],
                                             encT[:, 0, s0 * 32:(s0 + XCHE) * 32],
                                             start=True, stop=True)
                            dst = t[:, :, m * 32:(m + 1) * 32]
                            srcv = ps[:].rearrange("p (s b) -> p s b", b=32)
                            nc.vector.tensor_scalar(dst, srcv, XSC,
                                                    bias[:, m:m + 1],
                                                    ALU.mult, ALU.add)
                        units.append(unit)
                    return units

                q_enc = []
                for cwin in (0, 1):
                    for u in emit_xproj_e(0, cwin):
                        u()
                    for u in emit_xproj_e(1, NCH_E - 1 - cwin):
                        u()
                next_win = 2

                morder_e = (list(range(KE, 2 * KE)) + list(range(2 * KE, 3 * KE))
                            + list(range(0, KE)) + list(range(3 * KE, 4 * KE)))

                h_prev = None
                h0e = egsb.tile([128, 2 * KE * B], BF16, tag="h0e", name="h0e")
                nc.vector.tensor_copy(h0e[:, 0:KE * B], h0f[:])
                nc.vector.tensor_copy(h0e[:, KE * B:2 * KE * B], h0b[:])
                c_e = spool.tile([128, 2 * KE * B], F32, tag="c_e", name="c_e")
                nc.vector.memset(c_e[:], 0.0)
                HW_E = KE * B  # 64: one direction's h width
                for step in range(S):
                    if step % XCHE == 0 and next_win < NCH_E:
                        q_enc.extend(emit_xproj_e(0, next_win))
                        q_enc.extend(emit_xproj_e(1, NCH_E - 1 - next_win))
                        next_win += 1
                    for _ in range(2):
                        if q_enc:
                            q_enc.pop(0)()

                    s_f = step
                    s_b = S - 1 - step
                    xt_f = xch_f[s_f // XCHE]
                    xt_b = xch_b[s_b // XCHE]
                    hc = h0e if step == 0 else h_prev

                    ps = egps.tile([128, 2 * ME * 32], F32, tag="g", name="g")
                    ps4 = ps[:].rearrange("p (d g c) -> p d g c", d=2, g=4)
                    nc.tensor.matmul(ps[:, 0:ME * 32], ident[:, :],
                                     xt_f[:, s_f % XCHE, :],
                                     start=True, stop=False, skip_group_check=True)
                    nc.tensor.matmul(ps[:, ME * 32:2 * ME * 32], ident[:, :],
                                     xt_b[:, s_b % XCHE, :],
                                     start=False, stop=False, skip_group_check=True)
                    n_mm = 2 * len(morder_e) * KE
                    i_mm = 0
                    for m in morder_e:
                        for d in (0, 1):
                            whh_s = whh_f_s if d == 0 else whh_b_s
                            off = d * ME * 32 + m * 32
                            for k in range(KE):
                                rhs = hc[:, d * HW_E + k * B:d * HW_E + (k + 1) * B]
                                lt = whh_s[:, (k * ME + m) * 128:
                                           (k * ME + m + 1) * 128]
                                i_mm += 1
                                nc.tensor.matmul(
                                    ps[:, off:off + 32], lt, rhs,
                                    start=False, stop=(i_mm == n_mm),
                                    skip_group_check=True)
                    gf = egsb.tile([128, 2, GWE], F32, tag="gf", name="gf")
                    gg = egsb.tile([128, 2, GWE], F32, tag="gg", name="gg")
                    gi = egsb.tile([128, 2, GWE], F32, tag="gi", name="gi")
                    go = egsb.tile([128, 2, GWE], F32, tag="go", name="go")
                    t1 = egsb.tile([128, 2 * GWE], F32, tag="t1", name="t1")
                    t2 = egsb.tile([128, 2 * GWE], F32, tag="t2", name="t2")
                    tct = egsb.tile([128, 2 * GWE], F32, tag="tct", name="tct")
                    nc.scalar.activation(gf[:], ps4[:, :, 1, :], AF.Sigmoid,
                                         scale=GSC)
                    nc.scalar.activation(gg[:], ps4[:, :, 2, :], AF.Tanh,
                                         scale=GSC)
                    nc.scalar.activation(gi[:], ps4[:, :, 0, :], AF.Sigmoid,
                                         scale=GSC)
                    nc.scalar.activation(go[:], ps4[:, :, 3, :], AF.Sigmoid,
                                         scale=GSC)
                    gff = gf[:].rearrange("p d c -> p (d c)")
                    ggf = gg[:].rearrange("p d c -> p (d c)")
                    gif = gi[:].rearrange("p d c -> p (d c)")
                    gof = go[:].rearrange("p d c -> p (d c)")
                    nc.vector.tensor_tensor(t1[:], gff, c_e[:], ALU.mult)
                    nc.vector.tensor_tensor(t2[:], gif, ggf, ALU.mult)
                    nc.vector.tensor_tensor(c_e[:], t1[:], t2[:], ALU.add)
                    nc.scalar.activation(tct[:], c_e[:], AF.Tanh)
                    hn = egsb.tile([128, 2 * KE * B], BF16, tag="hn", name="hn")
                    nc.vector.tensor_tensor(hn[:], gof, tct[:], ALU.mult)
                    h_prev = hn
                    nc.gpsimd.tensor_copy(
                        memT4[:, 0:KE, :, s_f],
                        hn[:, 0:HW_E].rearrange("p (k b) -> p k b", k=KE))
                    nc.gpsimd.tensor_copy(
                        memT4[:, KE:2 * KE, :, s_b],
                        hn[:, HW_E:2 * HW_E].rearrange("p (k b) -> p k b", k=KE))

                while q_enc:
                    q_enc.pop(0)()

                if phases < 3:
                    raise _Stop

                # ---------- c_t (decoder c init) ----------
                ccT = egsb.tile([128, KD * B], BF16, tag="ccT", name="ccT")
                nc.vector.tensor_copy(ccT[:], c_e[:])
                with tc.tile_pool(name="ct_ps", bufs=2, space="PSUM") as cps, \
                     tc.tile_pool(name="ct_sb", bufs=2) as csb:
                    for m in range(KD):
                        ps = cps.tile([128, B], F32, tag="ct", name="ct")
                        for k in range(KD):
                            lt = wtr_s[:, (k * KD + m) * 128:(k * KD + m + 1) * 128]
                            nc.tensor.matmul(ps[:], lt, ccT[:, k * B:(k + 1) * B],
                                             start=(k == 0), stop=(k == KD - 1))
                        ab = csb.tile([128, B], F32, tag="ab", name="ab")
                        nc.scalar.activation(ab[:], ps[:], AF.Abs)
                        idt = csb.tile([128, B], F32, tag="idt", name="idt")
                        nc.scalar.activation(idt[:], ps[:], AF.Identity, scale=0.55)
                        nc.vector.scalar_tensor_tensor(
                            c_d[:, m * B:(m + 1) * B], ab[:],
                            0.45, idt[:], ALU.mult, ALU.add)

            if phases < 4:
                raise _Stop

            # ================= DECODER =================
            NCH_D = T // XCHD
            with tc.tile_pool(name="xd_ps", bufs=1, space="PSUM") as xdps, \
                 tc.tile_pool(name="xr_d", bufs=3) as xrd, \
                 tc.tile_pool(name="dg_ps", bufs=2, space="PSUM") as dgps, \
                 tc.tile_pool(name="dg_sb", bufs=2) as dgsb, \
                 tc.tile_pool(name="h_ring", bufs=3) as hpool, \
                 tc.tile_pool(name="at_ps", bufs=2, space="PSUM") as aps, \
                 tc.tile_pool(name="at_sb", bufs=2) as asb, \
                 tc.tile_pool(name="md_ps", bufs=1, space="PSUM") as mps, \
                 tc.tile_pool(name="md_sb", bufs=2) as msb, \
                 tc.tile_pool(name="cx_sb", bufs=1) as cxsb, \
                 tc.tile_pool(name="lg_ps", bufs=2, space="PSUM") as lps, \
                 tc.tile_pool(name="lg_sb", bufs=2) as lsb, \
                 tc.tile_pool(name="wf2_sb", bufs=2) as wfp:

                xch_d = {}

                def emit_xproj_d(c):
                    t = xrd.tile([128, XCHD, MD * 32], BF16, tag="xd", name="xd")
                    xch_d[c] = t
                    t0 = c * XCHD
                    units = []
                    for m in range(MD):
                        def unit(m=m, t=t, t0=t0):
                            ps = xdps.tile([128, XCHD * 32], F32, tag="xd",
                                           name="xd")
                            nc.tensor.matmul(ps[:],
                                             wih_d_s[:, m * 128:(m + 1) * 128],
                                             decT[:, 0, t0 * 32:(t0 + XCHD) * 32],
                                             start=True, stop=True)
                            dst = t[:, :, m * 32:(m + 1) * 32]
                            srcv = ps[:].rearrange("p (s b) -> p s b", b=32)
                            if m % 2 == 0:
                                nc.scalar.activation(dst, srcv, AF.Identity,
                                                     scale=XSC,
                                                     bias=bs_d_s[:, m:m + 1])
                            else:
                                nc.vector.tensor_scalar(dst, srcv, XSC,
                                                        bs_d_s[:, m:m + 1],
                                                        ALU.mult, ALU.add)
                        units.append(unit)
                    return units

                for cc in (0, 1):
                    for u in emit_xproj_d(cc):
                        u()
                next_xd = 2

                q_hi = []
                q_lo = []  # items: (chunk_tag, closure, pe_cost)

                def drain(n_hi=1, pe_budget=2600, upto_chunk=None):
                    while q_hi and n_hi > 0:
                        q_hi.pop(0)()
                        n_hi -= 1
                    if upto_chunk is not None:
                        while q_lo and q_lo[0][0] <= upto_chunk:
                            q_lo.pop(0)[1]()
                    spent = 0
                    while q_lo and spent < pe_budget:
                        tag, fn, cost = q_lo.pop(0)
                        fn()
                        spent += cost

                def memN_unit(b):
                    def fn():
                        for kd in range(2 * KE):
                            tp = aps.tile([128, 128], BF16, tag="a", name="tp",
                                          padded_shape=[128, 128])
                            nc.tensor.transpose(tp[:], memT4[:, kd, b, :],
                                                ident[:, :])
                            nc.vector.tensor_copy(
                                memN[:, b, kd * 128:(kd + 1) * 128], tp[:])
                    return fn
                for b in range(B):
                    q_lo.append((-1, memN_unit(b), 800))

                Hchunks = {}
                mid_ch = {}
                ctx_ch = {}
                wf2_cur = [None]

                NBG = NBG_  # b's per attention group (stacked psum rows at 0/32)

                def attn_unit(c, g):
                    def fn():
                        Hc = Hchunks[c]
                        if g == 0:
                            ctx_ch[c] = cxsb.tile([128, KD, BTC], BF16,
                                                  tag="ctx", name="ctx")
                        sc = aps.tile([NBG * TC, S], F32, tag="a", name="sc")
                        for j in range(NBG):
                            b = g * NBG + j
                            for k in range(2 * KE):
                                nc.tensor.matmul(sc[j * TC:(j + 1) * TC, :],
                                                 Hc[:, k, b, :],
                                                 memT4[:, k, b, :],
                                                 start=(k == 0),
                                                 stop=(k == 2 * KE - 1))
                        # softmax via sigmoid: e^x = sig(x)/(1-sig(x))
                        NP = NBG * TC
                        sg = asb.tile([NP, S], F32, tag="sg", name="sg")
                        nc.scalar.activation(sg[:], sc[:], AF.Sigmoid, scale=SCALE)
                        # om = 1 - sg on the scalar engine (Identity table)
                        om = asb.tile([NP, S], F32, tag="om", name="om")
                        nc.scalar.activation(om[:], sg[:], AF.Identity,
                                             scale=-1.0, bias=1.0)
                        orc = asb.tile([NP, S], F32, tag="orc", name="orc")
                        nc.vector.reciprocal(orc[:], om[:])
                        e = asb.tile([NP, S], F32, tag="e", name="e")
                        nc.vector.tensor_tensor(e[:], sg[:], orc[:], ALU.mult)
                        den = asb.tile([NP, 1], F32, tag="den", name="den")
                        nc.vector.tensor_reduce(den[:], e[:], AX.X, ALU.add)
                        rec = asb.tile([NP, 1], F32, tag="rec", name="rec")
                        nc.vector.reciprocal(rec[:], den[:])
                        p_sb = asb.tile([NP, S], BF16, tag="p", name="p")
                        nc.vector.tensor_scalar_mul(p_sb[:], e[:], rec[:])
                        for j in range(NBG):
                            b = g * NBG + j
                            pt = aps.tile([S, TC], BF16, tag="a", name="pt",
                                          padded_shape=[128, TC])
                            nc.tensor.transpose(
                                pt[:], p_sb[j * TC:(j + 1) * TC, :],
                                ident[j * TC:(j + 1) * TC, j * TC:(j + 1) * TC])
                            pt_sb = asb.tile([S, TC], BF16, tag="pts", name="pts")
                            nc.scalar.copy(pt_sb[:], pt[:])
                            cx = aps.tile([128, KD * TC], F32, tag="a", name="cx")
                            for kd in range(KD):
                                nc.tensor.matmul(
                                    cx[:, kd * TC:(kd + 1) * TC],
                                    memN[:, b, kd * 128:(kd + 1) * 128],
                                    pt_sb[:], start=True, stop=True)
                            nc.vector.tensor_copy(
                                ctx_ch[c][:, :, b * TC:(b + 1) * TC],
                                cx[:].rearrange("p (k t) -> p k t", k=KD))
                    return fn

                def ffn_unit(c, m, half):
                    def fn():
                        Hc = Hchunks[c]
                        if m == 0 and half == 0:
                            mid_ch[c] = msb.tile([128, KD, BTC], BF16,
                                                 tag="mid", name="mid")
                        HW2 = BTC // 2
                        ps = mps.tile([128, HW2], F32, tag="md", name="md")
                        Hflat = Hc[:].rearrange("p k b t -> p (k b t)")
                        for k in range(KD):
                            lt = wf1_s[:, (k * KD + m) * 128:
                                       (k * KD + m + 1) * 128]
                            nc.tensor.matmul(ps[:], lt,
                                             Hflat[:, k * BTC + half * HW2:
                                                   k * BTC + (half + 1) * HW2],
                                             start=(k == 0), stop=False)
                        ctxf = ctx_ch[c][:].rearrange("p k t -> p (k t)")
                        for k in range(KD):
                            kk = KD + k
                            lt = wf1_s[:, (kk * KD + m) * 128:
                                       (kk * KD + m + 1) * 128]
                            nc.tensor.matmul(ps[:], lt,
                                             ctxf[:, k * BTC + half * HW2:
                                                  k * BTC + (half + 1) * HW2],
                                             start=False, stop=(k == KD - 1))
                        ab = asb.tile([128, HW2], F32, tag="mab", name="mab")
                        nc.scalar.activation(ab[:], ps[:], AF.Abs,
                                             bias=b1a_s[:, m:m + 1])
                        idt = asb.tile([128, HW2], F32, tag="midt", name="midt")
                        nc.scalar.activation(idt[:], ps[:], AF.Identity,
                                             scale=0.55, bias=b1h_s[:, m:m + 1])
                        nc.vector.scalar_tensor_tensor(
                            mid_ch[c][:, m, half * HW2:(half + 1) * HW2],
                            ab[:], 0.45, idt[:], ALU.mult, ALU.add)
                    return fn

                def wf2_unit(vc):
                    def fn():
                        t = wfp.tile([128, KD, 500], BF16, tag="wf2c", name="wf2c")
                        nc.sync.dma_start(t[:],
                                          wf2_3d[:, :, vc * 500:(vc + 1) * 500])
                        wf2_cur[0] = t
                    return fn

                def vocab_unit(c, vc, grp):
                    def fn():
                        wt = wf2_cur[0]
                        midf = mid_ch[c][:].rearrange("p k t -> p (k t)")
                        ps = lps.tile([128, 500], F32, tag="lg", name="lg")
                        for k in range(KD):
                            lt = midf[:, k * BTC + grp * 128:
                                      k * BTC + (grp + 1) * 128]
                            nc.tensor.matmul(ps[:], lt, wt[:, k, :],
                                             start=(k == 0), stop=(k == KD - 1))
                        lo = lsb.tile([128, 500], BF16, tag="lo", name="lo")
                        if grp % 2 == 0:
                            nc.scalar.copy(lo[:], ps[:])
                        else:
                            nc.vector.tensor_copy(lo[:], ps[:])
                        nb = 128 // TC
                        b0 = grp * nb
                        for bl in range(nb):
                            nc.sync.dma_start(
                                out.ap()[b0 + bl, c * TC:(c + 1) * TC,
                                         vc * 500:(vc + 1) * 500],
                                lo[bl * TC:(bl + 1) * TC, :])
                    return fn

                def queue_chunk_work(c):
                    for g in range(B // NBG):
                        q_lo.append((c, attn_unit(c, g), 800))
                    for m in range(KD):
                        for half in range(2):
                            q_lo.append((c, ffn_unit(c, m, half), 1750))
                    for vc in range(NVC):
                        q_lo.append((c, wf2_unit(vc), 0))
                        for grp in range(BTC // 128):
                            q_lo.append((c, vocab_unit(c, vc, grp), 900))

                morder_d = (list(range(KD, 2 * KD)) + list(range(2 * KD, 3 * KD))
                            + list(range(0, KD)) + list(range(3 * KD, 4 * KD)))

                for t in range(T):
                    tci = t // TC
                    t_loc = t % TC
                    if t_loc == 0:
                        if tci >= 3:
                            drain(n_hi=0, pe_budget=0, upto_chunk=tci - 3)
                        Hchunks[tci] = hpool.tile([128, KD, B, TC], BF16,
                                                  tag="H", name="H")
                    if t % XCHD == 0 and next_xd < NCH_D:
                        q_hi.extend(emit_xproj_d(next_xd))
                        next_xd += 1

                    if phases >= 6:
                        drain(n_hi=2, pe_budget=PE_BUDGET)
                    else:
                        drain(n_hi=2, pe_budget=0)

                    xt = xch_d[t // XCHD]
                    ps = dgps.tile([128, MD * 32], F32, tag="gd", name="gd")
                    nc.tensor.matmul(ps[:], ident[:, :], xt[:, t % XCHD, :],
                                     start=True, stop=False, skip_group_check=True)
                    for mi, m in enumerate(morder_d):
                        for k in range(KD):
                            if t == 0:
                                rhs = h0d[:, k * B:(k + 1) * B]
                            else:
                                rhs = h_prev_d[:, k * B:(k + 1) * B]
                            lt = whh_d_s[:, (k * MD + m) * 128:
                                         (k * MD + m + 1) * 128]
                            nc.tensor.matmul(
                                ps[:, m * 32:(m + 1) * 32], lt, rhs,
                                start=False,
                                stop=(mi == len(morder_d) - 1 and k == KD - 1),
                                skip_group_check=True)
                    gfd = dgsb.tile([128, GWD], F32, tag="gf", name="gf")
                    ggd = dgsb.tile([128, GWD], F32, tag="gg", name="gg")
                    gid = dgsb.tile([128, GWD], F32, tag="gi", name="gi")
                    god = dgsb.tile([128, GWD], F32, tag="go", name="go")
                    t1d = dgsb.tile([128, GWD], F32, tag="t1", name="t1")
                    t2d = dgsb.tile([128, GWD], F32, tag="t2", name="t2")
                    tctd = dgsb.tile([128, GWD], F32, tag="tct", name="tct")
                    nc.scalar.activation(gfd[:], ps[:, GWD:2 * GWD], AF.Sigmoid,
                                         scale=GSC)
                    nc.scalar.activation(ggd[:], ps[:, 2 * GWD:3 * GWD], AF.Tanh,
                                         scale=GSC)
                    nc.scalar.activation(gid[:], ps[:, 0:GWD], AF.Sigmoid,
                                         scale=GSC)
                    nc.scalar.activation(god[:], ps[:, 3 * GWD:4 * GWD], AF.Sigmoid,
                                         scale=GSC)
                    nc.vector.tensor_tensor(t1d[:], gfd[:], c_d[:], ALU.mult)
                    nc.vector.tensor_tensor(t2d[:], gid[:], ggd[:], ALU.mult)
                    nc.vector.tensor_tensor(c_d[:], t1d[:], t2d[:], ALU.add)
                    nc.scalar.activation(tctd[:], c_d[:], AF.Tanh)
                    hn = dgsb.tile([128, KD * B], BF16, tag="hn", name="hn")
                    nc.vector.tensor_tensor(hn[:], god[:], tctd[:], ALU.mult)
                    h_prev_d = hn
                    nc.gpsimd.tensor_copy(
                        Hchunks[tci][:, :, :, t_loc],
                        hn[:].rearrange("p (k b) -> p k b", k=KD))

                    if phases >= 5 and t_loc == TC - 1:
                        queue_chunk_work(tci)

                if phases >= 5:
                    while q_hi:
                        q_hi.pop(0)()
                    while q_lo:
                        q_lo.pop(0)[1]()
        except _Stop:
            pass

    nc.compile()
    return nc


def prep_inputs(i, S=128, T=128, V=32000, VS=4000):
    """Host-side input staging -> list of 8 per-core in_maps."""
    def as_np(x, dt=np.float32):
        return np.ascontiguousarray(np.asarray(x), dtype=dt)

    tok = as_np(i["tok_emb"]).astype(bf16)

    def idx_prep(flat):
        a = flat.astype(np.int16).reshape(-1, 16).T
        return np.ascontiguousarray(np.tile(a, (8, 1)))

    inp = as_np(i["inp"], np.int64)
    x = as_np(i["x"], np.int64)
    idx_e = idx_prep(inp.T.reshape(-1))              # s-major: s*32+b
    dmat = np.zeros((B, T), np.int64)
    dmat[:, 1:] = x[:, :T - 1]
    idx_d = idx_prep(dmat.T.reshape(-1))             # t-major: t*32+b

    startT = as_np(i["start_emb"]).reshape(D_EMB, 1).astype(bf16)

    est = as_np(i["enc_style_emb"])                  # [2, 512]
    diff_e = (est[1] - est[0]).reshape(1, -1).astype(bf16)
    e0T = np.ascontiguousarray(est[0].reshape(KD, 128).T)
    sty = as_np(i["style_emb"])                      # [2, 512]
    diff_s = (sty[1] - sty[0]).reshape(1, -1).astype(bf16)
    s0T = np.ascontiguousarray(sty[0].reshape(KD, 128).T)
    lab_i = as_np(i["label_i"], np.float32).reshape(1, B).astype(bf16)
    lab_d = as_np(i["label"], np.float32).reshape(1, B).astype(bf16)

    def wihT(w, nm):
        a = w.reshape(nm, 128, 128)
        return np.ascontiguousarray(a.transpose(2, 0, 1).reshape(128, nm * 128)
                                    ).astype(bf16)

    def whhT(w, nk, nm):
        a = w.reshape(nm, 128, nk, 128)
        a = a.transpose(3, 2, 0, 1)
        return np.ascontiguousarray(a.reshape(128, nk * nm * 128)).astype(bf16)

    wih_f = wihT(as_np(i["Wih_f"]), ME)
    wih_b = wihT(as_np(i["Wih_b"]), ME)
    wih_d = wihT(as_np(i["Wih_d"]), MD)
    if FP8:
        fp8 = ml_dtypes.float8_e4m3fn
        whh_f = (whhT(as_np(i["Whh_f"]), KE, ME).astype(np.float32) * 64).astype(fp8)
        whh_b = (whhT(as_np(i["Whh_b"]), KE, ME).astype(np.float32) * 64).astype(fp8)
        whh_d = (whhT(as_np(i["Whh_d"]), KD, MD).astype(np.float32) * 64).astype(fp8)
    else:
        whh_f = whhT(as_np(i["Whh_f"]), KE, ME)
        whh_b = whhT(as_np(i["Whh_b"]), KE, ME)
        whh_d = whhT(as_np(i["Whh_d"]), KD, MD)
    wtr = whhT(as_np(i["W_tr"]), KD, KD)
    wf1 = whhT(as_np(i["W_f1"]), 8, KD)

    wf2_full = as_np(i["W_f2"])                      # [V, 512]

    def bcol(v, nm):
        return np.ascontiguousarray(v.reshape(nm, 128).T)

    _xs = 64.0 if FP8 else 1.0
    bs_f = bcol(as_np(i["bih_f"]) + as_np(i["bhh_f"]), ME) * _xs
    bs_b = bcol(as_np(i["bih_b"]) + as_np(i["bhh_b"]), ME) * _xs
    bs_d = bcol(as_np(i["bih_d"]) + as_np(i["bhh_d"]), MD) * _xs
    b1 = as_np(i["b_f1"])
    b1a = bcol(b1, KD)
    b1h = bcol(0.55 * b1, KD)

    common = dict(tokb=tok, idx_e=idx_e, idx_d=idx_d, startT=startT,
                  diff_e=diff_e, e0T=e0T, lab_i=lab_i,
                  diff_s=diff_s, s0T=s0T, lab_d=lab_d,
                  wih_f=wih_f, wih_b=wih_b, wih_d=wih_d,
                  whh_f=whh_f, whh_b=whh_b, whh_d=whh_d,
                  wtr=wtr, wf1=wf1,
                  bs_f=bs_f, bs_b=bs_b, bs_d=bs_d, b1a=b1a, b1h=b1h)
    in_maps = []
    for c in range(N_CORES):
        shard = wf2_full[c * VS:(c + 1) * VS]        # [VS, 512]
        a = shard.reshape(VS, KD, 128)
        wf2c = np.ascontiguousarray(a.transpose(2, 1, 0).reshape(128, KD * VS)
                                    ).astype(bf16)
        in_maps.append(dict(common, wf2=wf2c))
    return in_maps


_NC_CACHE = {}


def kernel(**inputs):
    key = "full"
    if key not in _NC_CACHE:
        _NC_CACHE[key] = build()
    nc = _NC_CACHE[key]
    in_maps = prep_inputs(inputs)
    res = run_bass_kernel_spmd(nc, in_maps, core_ids=list(range(N_CORES)))
    full = np.concatenate([r["out"] for r in res.results], axis=2)
    return full.astype(np.float32)


# revision 24
# speedup vs baseline: 1.2140x; 1.0312x over previous
"""DenoiseLSTM Trainium2 kernel (8 NeuronCores, SPMD), v2.

Structure (single fused program per core; recurrences replicated, vocab
projection sharded over V=32000 -> 4000 cols/core):

  A. weight loads + token gathers + h0 inits.
  B. encoder (fwd+bwd interleaved). Input projections are computed into
     SBUF ring chunks (no DRAM roundtrip) and PRELOADED into the gate
     PSUM via an identity matmul, so the recurrent matmuls accumulate on
     top (start=False) and no vector add sits on the critical path. The
     final gate multiply writes h straight into mem_T (no copy); the
     next step's matmul reads it back strided.
  C. decoder, same gate scheme, h written into an H ring chunk.
     Attention + FFN + vocab projection for each completed 32-step
     t-chunk are emitted into the gaps between decoder steps via a work
     queue, so the big parallel matmuls fill the PE idle time of the
     serial recurrence. Logits are written bf16; the host upcasts.
"""
import sys

sys.path.insert(0, "/opt/trn_rl_repo")

from contextlib import ExitStack

import numpy as np
import ml_dtypes

import concourse.bass as bass
import concourse.bacc as bacc
import concourse.mybir as mybir
import concourse.tile as tile
from concourse.bass_utils import run_bass_kernel_spmd
from concourse.masks import make_identity

bf16 = ml_dtypes.bfloat16
F32 = mybir.dt.float32
BF16 = mybir.dt.bfloat16
I16 = mybir.dt.int16
AF = mybir.ActivationFunctionType
ALU = mybir.AluOpType
AX = mybir.AxisListType

B = 32
D_EMB = 128
D_ENC = 256
D_DEC = 512
N_CORES = 8
KE = D_ENC // 128      # 2
ME = 4 * D_ENC // 128  # 8
KD = D_DEC // 128      # 4
MD = 4 * D_DEC // 128  # 16
GWE = KE * 32          # 64  (one gate group width, encoder)
GWD = KD * 32          # 128 (one gate group width, decoder)


FP8 = False


class _Stop(Exception):
    pass


def build(S=128, T=128, V=32000, VS=4000, phases=9,
          TC=32,      # attention/FFN/vocab t-chunk
          XCHE=16,    # encoder xproj ring chunk (steps)
          XCHD=8,     # decoder xproj ring chunk (steps)
          PE_BUDGET=3200, NBG_=2, FP8_GATES=None,
          ):
    NI_E = B * S
    NI_D = B * T
    NTC = T // TC
    NVC = VS // 500
    BTC = B * TC              # columns per t-chunk (b-major: b*TC+ti)
    SCALE = 1.0 / float(np.sqrt(np.float32(2 * D_ENC)))

    nc = bacc.Bacc("TRN2", target_bir_lowering=False, debug=False)

    # ---- external inputs ----
    tokb = nc.dram_tensor("tokb", [V, D_EMB], BF16, kind="ExternalInput")
    idx_e = nc.dram_tensor("idx_e", [128, NI_E // 16], I16, kind="ExternalInput")
    idx_d = nc.dram_tensor("idx_d", [128, NI_D // 16], I16, kind="ExternalInput")
    startT = nc.dram_tensor("startT", [128, 1], BF16, kind="ExternalInput")
    diff_e = nc.dram_tensor("diff_e", [1, 2 * D_ENC], BF16, kind="ExternalInput")
    e0T = nc.dram_tensor("e0T", [128, KD], F32, kind="ExternalInput")
    lab_i = nc.dram_tensor("lab_i", [1, B], BF16, kind="ExternalInput")
    diff_s = nc.dram_tensor("diff_s", [1, D_DEC], BF16, kind="ExternalInput")
    s0T = nc.dram_tensor("s0T", [128, KD], F32, kind="ExternalInput")
    lab_d = nc.dram_tensor("lab_d", [1, B], BF16, kind="ExternalInput")
    wih_f = nc.dram_tensor("wih_f", [128, ME * 128], BF16, kind="ExternalInput")
    wih_b = nc.dram_tensor("wih_b", [128, ME * 128], BF16, kind="ExternalInput")
    wih_d = nc.dram_tensor("wih_d", [128, MD * 128], BF16, kind="ExternalInput")
    if FP8_GATES is None:
        FP8_GATES = FP8
    WDT = mybir.dt.float8e4 if FP8_GATES else BF16
    GSC = 1.0 / 64.0 if FP8_GATES else 1.0
    XSC = 64.0 if FP8_GATES else 1.0
    whh_f = nc.dram_tensor("whh_f", [128, KE * ME * 128], WDT, kind="ExternalInput")
    whh_b = nc.dram_tensor("whh_b", [128, KE * ME * 128], WDT, kind="ExternalInput")
    whh_d = nc.dram_tensor("whh_d", [128, KD * MD * 128], WDT, kind="ExternalInput")
    wtr = nc.dram_tensor("wtr", [128, KD * KD * 128], BF16, kind="ExternalInput")
    wf1 = nc.dram_tensor("wf1", [128, 8 * KD * 128], BF16, kind="ExternalInput")
    wf2 = nc.dram_tensor("wf2", [128, KD * VS], BF16, kind="ExternalInput")
    bs_f = nc.dram_tensor("bs_f", [128, ME], F32, kind="ExternalInput")
    bs_b = nc.dram_tensor("bs_b", [128, ME], F32, kind="ExternalInput")
    bs_d = nc.dram_tensor("bs_d", [128, MD], F32, kind="ExternalInput")
    b1a = nc.dram_tensor("b1a", [128, KD], F32, kind="ExternalInput")
    b1h = nc.dram_tensor("b1h", [128, KD], F32, kind="ExternalInput")

    out = nc.dram_tensor("out", [B, T, VS], BF16, kind="ExternalOutput")
    dbg_mem = nc.dram_tensor("dbg_mem", [128, 2 * KE * B * S], BF16,
                             kind="ExternalOutput")
    dbg_cd = nc.dram_tensor("dbg_cd", [128, KD * B], F32, kind="ExternalOutput")
    dbg_h = nc.dram_tensor("dbg_h", [128, KD * B * T], BF16, kind="ExternalOutput")
    wf2_3d = wf2.ap().rearrange("p (k v) -> p k v", k=KD)

    with tile.TileContext(nc) as tc, ExitStack() as ctx:
        # ---------------- persistent pools ----------------
        wpool = ctx.enter_context(tc.tile_pool(name="weights", bufs=1))
        spool = ctx.enter_context(tc.tile_pool(name="state", bufs=1))
        big = ctx.enter_context(tc.tile_pool(name="big", bufs=1))

        def load(dram, shape, dtype, tag, pool=None):
            t = (pool or wpool).tile(shape, dtype, tag=tag, name=tag)
            nc.sync.dma_start(t[:], dram[:, :])
            return t

        wih_d_s = load(wih_d, [128, MD * 128], BF16, "wih_d")
        bs_d_s = load(bs_d, [128, MD], F32, "bs_d")
        b1a_s = load(b1a, [128, KD], F32, "b1a")
        b1h_s = load(b1h, [128, KD], F32, "b1h")
        startT_s = load(startT, [128, 1], BF16, "startT")
        e0T_s = load(e0T, [128, KD], F32, "e0T")
        s0T_s = load(s0T, [128, KD], F32, "s0T")
        ident = wpool.tile([128, 128], BF16, tag="ident", name="ident")
        make_identity(nc, ident)

        diff_e_s = wpool.tile([1, 2 * D_ENC], BF16, tag="diff_e", name="diff_e")
        nc.sync.dma_start(diff_e_s[:], diff_e[:, :])
        diff_s_s = wpool.tile([1, D_DEC], BF16, tag="diff_s", name="diff_s")
        nc.sync.dma_start(diff_s_s[:], diff_s[:, :])
        lab_i_s = wpool.tile([1, B], BF16, tag="lab_i", name="lab_i")
        nc.sync.dma_start(lab_i_s[:], lab_i[:, :])
        lab_d_s = wpool.tile([1, B], BF16, tag="lab_d", name="lab_d")
        nc.sync.dma_start(lab_d_s[:], lab_d[:, :])

        whh_d_s = load(whh_d, [128, KD * MD * 128], WDT, "whh_d")
        wf1_s = load(wf1, [128, 8 * KD * 128], BF16, "wf1")

        # ---------------- gathers ----------------
        idx_e_s = wpool.tile([128, NI_E // 16], I16, tag="idx_e", name="idx_e")
        nc.sync.dma_start(idx_e_s[:], idx_e[:, :])
        idx_d_s = wpool.tile([128, NI_D // 16], I16, tag="idx_d", name="idx_d")
        nc.sync.dma_start(idx_d_s[:], idx_d[:, :])
        decT = big.tile([128, 1, NI_D], BF16, tag="decT", name="decT")  # cols t*32+b
        nc.gpsimd.dma_gather(decT[:], tokb[:, :], idx_d_s[:], NI_D, NI_D,
                             D_EMB, transpose=True, single_packet=False)
        nc.vector.tensor_copy(
            decT[:, 0, :].rearrange("p (b t) -> p b t", t=T)[:, :, 0],
            startT_s[:, 0:1].to_broadcast((128, B)))

        # ---------------- big state tensors ----------------
        mem_T = big.tile([128, 2 * KE, B, S], BF16, tag="mem_T", name="mem_T")
        memT4 = mem_T[:]
        memN = big.tile([128, B, 2 * KE * 128], BF16, tag="memN", name="memN")

        # ---------------- init h0 / c ----------------
        h0f = spool.tile([128, KE * B], BF16, tag="h0f", name="h0f")
        h0b = spool.tile([128, KE * B], BF16, tag="h0b", name="h0b")
        h0d = spool.tile([128, KD * B], BF16, tag="h0d", name="h0d")
        c_f = spool.tile([128, KE * B], F32, tag="c_f", name="c_f")
        c_b = spool.tile([128, KE * B], F32, tag="c_b", name="c_b")
        c_d = spool.tile([128, KD * B], F32, tag="c_d", name="c_d")
        nc.vector.memset(c_f[:], 0.0)
        nc.vector.memset(c_b[:], 0.0)

        with tc.tile_pool(name="init_ps", bufs=2, space="PSUM") as ips, \
             tc.tile_pool(name="init_sb", bufs=2) as isb:
            for dst, dbase in ((h0f, 0), (h0b, KE)):
                for k in range(KE):
                    ps = ips.tile([128, B], F32, tag="i", name="i")
                    col = (dbase + k) * 128
                    nc.tensor.matmul(ps[:], diff_e_s[:, col:col + 128],
                                     lab_i_s[:, :], start=True, stop=True)
                    f32t = isb.tile([128, B], F32, tag="h0t", name="h0t")
                    nc.vector.tensor_scalar_add(f32t[:], ps[:],
                                                e0T_s[:, dbase + k:dbase + k + 1])
                    nc.vector.tensor_copy(dst[:, k * B:(k + 1) * B], f32t[:])
            for k in range(KD):
                ps = ips.tile([128, B], F32, tag="i", name="i")
                nc.tensor.matmul(ps[:], diff_s_s[:, k * 128:(k + 1) * 128],
                                 lab_d_s[:, :], start=True, stop=True)
                f32t = isb.tile([128, B], F32, tag="h0t", name="h0t")
                nc.vector.tensor_scalar_add(f32t[:], ps[:], s0T_s[:, k:k + 1])
                nc.vector.tensor_copy(h0d[:, k * B:(k + 1) * B], f32t[:])

        try:
            if phases < 2:
                raise _Stop

            # ================= ENCODER =================
            NCH_E = S // XCHE
            with tc.tile_pool(name="encw", bufs=1) as encw, \
                 tc.tile_pool(name="xe_ps", bufs=2, space="PSUM") as xeps, \
                 tc.tile_pool(name="xr_f", bufs=3) as xrf, \
                 tc.tile_pool(name="xr_b", bufs=3) as xrb, \
                 tc.tile_pool(name="eg_ps", bufs=2, space="PSUM") as egps, \
                 tc.tile_pool(name="eg_sb", bufs=2) as egsb:

                wih_f_s = load(wih_f, [128, ME * 128], BF16, "wih_f", pool=encw)
                wih_b_s = load(wih_b, [128, ME * 128], BF16, "wih_b", pool=encw)
                whh_f_s = load(whh_f, [128, KE * ME * 128], WDT, "whh_f", pool=encw)
                whh_b_s = load(whh_b, [128, KE * ME * 128], WDT, "whh_b", pool=encw)
                wtr_s = load(wtr, [128, KD * KD * 128], BF16, "wtr", pool=encw)
                bs_f_s = load(bs_f, [128, ME], F32, "bs_f", pool=encw)
                bs_b_s = load(bs_b, [128, ME], F32, "bs_b", pool=encw)
                encT = encw.tile([128, 1, NI_E], BF16, tag="encT", name="encT")
                nc.gpsimd.dma_gather(encT[:], tokb[:, :], idx_e_s[:], NI_E, NI_E,
                                     D_EMB, transpose=True, single_packet=False)

                xch_f = {}
                xch_b = {}

                encT3 = encT[:, 0, :].rearrange("p (b s) -> p b s", s=S)

                def emit_xproj_e(d, c):
                    pool, store, w_s, bias = ((xrf, xch_f, wih_f_s, bs_f_s) if d == 0
                                              else (xrb, xch_b, wih_b_s, bs_b_s))
                    # ring layout [128, m, b, s]: casts contiguous per m
                    t = pool.tile([128, ME, B, XCHE], BF16, tag=f"x{d}",
                                  name=f"x{d}")
                    store[c] = t
                    s0 = c * XCHE
                    units = []
                    for m in range(ME):
                        def unit(m=m, t=t, s0=s0, w_s=w_s, bias=bias):
                            ps = xeps.tile([128, B * XCHE], F32, tag="xe", name="xe")
                            nc.tensor.matmul(ps[:], w_s[:, m * 128:(m + 1) * 128],
                                             encT3[:, :, s0:s0 + XCHE],
                                             start=True, stop=True)
                            nc.vector.tensor_scalar(
                                t[:, m, :, :],
                                ps[:].rearrange("p (b s) -> p b s", s=XCHE),
                                XSC, bias[:, m:m + 1], ALU.mult, ALU.add)
                        units.append(unit)
                    return units

                q_enc = []
                for cwin in (0, 1):
                    for u in emit_xproj_e(0, cwin):
                        u()
                    for u in emit_xproj_e(1, NCH_E - 1 - cwin):
                        u()
                next_win = 2

                morder_e = (list(range(KE, 2 * KE)) + list(range(2 * KE, 3 * KE))
                            + list(range(0, KE)) + list(range(3 * KE, 4 * KE)))

                h_prev = {0: h0f, 1: h0b}
                GPO = {0: 0, 1: 1, 2: 3, 3: 2}  # gate -> region (i,f,o,g order)
                for step in range(S):
                    if step % XCHE == 0 and next_win < NCH_E:
                        q_enc.extend(emit_xproj_e(0, next_win))
                        q_enc.extend(emit_xproj_e(1, NCH_E - 1 - next_win))
                        next_win += 1
                    for _ in range(2):
                        if q_enc:
                            q_enc.pop(0)()

                    for d in (0, 1):
                        s_in = step if d == 0 else S - 1 - step
                        xt = (xch_f if d == 0 else xch_b)[s_in // XCHE]
                        whh_s = whh_f_s if d == 0 else whh_b_s
                        kk0 = 0 if d == 0 else KE
                        cst = c_f if d == 0 else c_b
                        hc = h_prev[d]
                        s_loc = s_in % XCHE

                        ps = egps.tile([128, ME * 32], F32, tag=f"g{d}",
                                       name=f"g{d}")
                        # preload x-projection (+bias): rhs strided (m,b) cols
                        nc.tensor.matmul(
                            ps[:], ident[:, :], xt[:, :, :, s_loc]
                            .rearrange("p m b -> p (m b)"),
                            start=True, stop=False, skip_group_check=True)
                        n_mm = len(morder_e) * KE
                        i_mm = 0
                        for m in morder_e:
                            off = GPO[m // KE] * GWE + (m % KE) * 32
                            for k in range(KE):
                                rhs = hc[:, k * B:(k + 1) * B]
                                lt = whh_s[:, (k * ME + m) * 128:
                                           (k * ME + m + 1) * 128]
                                i_mm += 1
                                nc.tensor.matmul(
                                    ps[:, off:off + 32], lt, rhs,
                                    start=False, stop=(i_mm == n_mm),
                                    skip_group_check=True)
                        gio = egsb.tile([128, 3 * GWE], F32, tag=f"gio{d}",
                                        name=f"gio{d}")
                        gg = egsb.tile([128, GWE], F32, tag=f"gg{d}",
                                       name=f"gg{d}")
                        t1 = egsb.tile([128, GWE], F32, tag=f"t1{d}", name=f"t1{d}")
                        t2 = egsb.tile([128, GWE], F32, tag=f"t2{d}", name=f"t2{d}")
                        tct = egsb.tile([128, GWE], F32, tag=f"tc{d}",
                                        name=f"tc{d}")
                        nc.scalar.activation(gio[:], ps[:, 0:3 * GWE], AF.Sigmoid,
                                             scale=GSC)
                        nc.scalar.activation(gg[:], ps[:, 3 * GWE:4 * GWE],
                                             AF.Tanh, scale=GSC)
                        nc.vector.tensor_tensor(t1[:], gio[:, GWE:2 * GWE],
                                                cst[:], ALU.mult)
                        nc.vector.tensor_tensor(t2[:], gio[:, 0:GWE], gg[:],
                                                ALU.mult)
                        nc.vector.tensor_tensor(cst[:], t1[:], t2[:], ALU.add)
                        nc.scalar.activation(tct[:], cst[:], AF.Tanh)
                        hn = egsb.tile([128, KE * B], BF16, tag=f"hn{d}",
                                       name=f"hn{d}")
                        nc.vector.tensor_tensor(hn[:], gio[:, 2 * GWE:3 * GWE],
                                                tct[:], ALU.mult)
                        h_prev[d] = hn
                        nc.gpsimd.tensor_copy(
                            memT4[:, kk0:kk0 + KE, :, s_in],
                            hn[:].rearrange("p (k b) -> p k b", k=KE))

                while q_enc:
                    q_enc.pop(0)()

                if phases < 3:
                    raise _Stop

                # ---------- c_t (decoder c init) ----------
                ccT = egsb.tile([128, KD * B], BF16, tag="ccT", name="ccT")
                nc.vector.tensor_copy(ccT[:, 0:KE * B], c_f[:])
                nc.vector.tensor_copy(ccT[:, KE * B:2 * KE * B], c_b[:])
                with tc.tile_pool(name="ct_ps", bufs=2, space="PSUM") as cps, \
                     tc.tile_pool(name="ct_sb", bufs=2) as csb:
                    for m in range(KD):
                        ps = cps.tile([128, B], F32, tag="ct", name="ct")
                        for k in range(KD):
                            lt = wtr_s[:, (k * KD + m) * 128:(k * KD + m + 1) * 128]
                            nc.tensor.matmul(ps[:], lt, ccT[:, k * B:(k + 1) * B],
                                             start=(k == 0), stop=(k == KD - 1))
                        ab = csb.tile([128, B], F32, tag="ab", name="ab")
                        nc.scalar.activation(ab[:], ps[:], AF.Abs)
                        idt = csb.tile([128, B], F32, tag="idt", name="idt")
                        nc.scalar.activation(idt[:], ps[:], AF.Identity, scale=0.55)
                        nc.vector.scalar_tensor_tensor(
                            c_d[:, m * B:(m + 1) * B], ab[:],
                            0.45, idt[:], ALU.mult, ALU.add)

            if phases < 4:
                raise _Stop

            # ================= DECODER =================
            NCH_D = T // XCHD
            with tc.tile_pool(name="xd_ps", bufs=1, space="PSUM") as xdps, \
                 tc.tile_pool(name="xr_d", bufs=3) as xrd, \
                 tc.tile_pool(name="dg_ps", bufs=2, space="PSUM") as dgps, \
                 tc.tile_pool(name="dg_sb", bufs=2) as dgsb, \
                 tc.tile_pool(name="h_ring", bufs=3) as hpool, \
                 tc.tile_pool(name="at_ps", bufs=2, space="PSUM") as aps, \
                 tc.tile_pool(name="at_sb", bufs=2) as asb, \
                 tc.tile_pool(name="md_ps", bufs=1, space="PSUM") as mps, \
                 tc.tile_pool(name="md_sb", bufs=2) as msb, \
                 tc.tile_pool(name="cx_sb", bufs=1) as cxsb, \
                 tc.tile_pool(name="lg_ps", bufs=2, space="PSUM") as lps, \
                 tc.tile_pool(name="lg_sb", bufs=2) as lsb, \
                 tc.tile_pool(name="wf2_sb", bufs=2) as wfp:

                xch_d = {}

                decT3 = decT[:, 0, :].rearrange("p (b t) -> p b t", t=T)

                def emit_xproj_d(c):
                    t = xrd.tile([128, MD, B, XCHD], BF16, tag="xd", name="xd")
                    xch_d[c] = t
                    t0 = c * XCHD
                    units = []
                    for m in range(MD):
                        def unit(m=m, t=t, t0=t0):
                            ps = xdps.tile([128, B * XCHD], F32, tag="xd",
                                           name="xd")
                            nc.tensor.matmul(ps[:],
                                             wih_d_s[:, m * 128:(m + 1) * 128],
                                             decT3[:, :, t0:t0 + XCHD],
                                             start=True, stop=True)
                            dst = t[:, m, :, :]
                            srcv = ps[:].rearrange("p (b s) -> p b s", s=XCHD)
                            if m % 2 == 0:
                                nc.scalar.activation(dst, srcv, AF.Identity,
                                                     scale=XSC,
                                                     bias=bs_d_s[:, m:m + 1])
                            else:
                                nc.vector.tensor_scalar(dst, srcv, XSC,
                                                        bs_d_s[:, m:m + 1],
                                                        ALU.mult, ALU.add)
                        units.append(unit)
                    return units

                for cc in (0, 1):
                    for u in emit_xproj_d(cc):
                        u()
                next_xd = 2

                q_hi = []
                q_lo = []  # items: (chunk_tag, closure, pe_cost)

                def drain(n_hi=1, pe_budget=2600, upto_chunk=None):
                    while q_hi and n_hi > 0:
                        q_hi.pop(0)()
                        n_hi -= 1
                    if upto_chunk is not None:
                        while q_lo and q_lo[0][0] <= upto_chunk:
                            q_lo.pop(0)[1]()
                    spent = 0
                    while q_lo and spent < pe_budget:
                        tag, fn, cost = q_lo.pop(0)
                        fn()
                        spent += cost

                def memN_unit(b):
                    def fn():
                        for kd in range(2 * KE):
                            tp = aps.tile([128, 128], BF16, tag="a", name="tp",
                                          padded_shape=[128, 128])
                            nc.tensor.transpose(tp[:], memT4[:, kd, b, :],
                                                ident[:, :])
                            nc.vector.tensor_copy(
                                memN[:, b, kd * 128:(kd + 1) * 128], tp[:])
                    return fn
                for b in range(B):
                    q_lo.append((-1, memN_unit(b), 800))

                Hchunks = {}
                mid_ch = {}
                ctx_ch = {}
                wf2_cur = [None]

                NBG = NBG_  # b's per attention group (stacked psum rows at 0/32)

                def attn_unit(c, g):
                    def fn():
                        Hc = Hchunks[c]
                        if g == 0:
                            ctx_ch[c] = cxsb.tile([128, KD, BTC], BF16,
                                                  tag="ctx", name="ctx")
                        sc = aps.tile([NBG * TC, S], F32, tag="a", name="sc")
                        for j in range(NBG):
                            b = g * NBG + j
                            for k in range(2 * KE):
                                nc.tensor.matmul(sc[j * TC:(j + 1) * TC, :],
                                                 Hc[:, k, b, :],
                                                 memT4[:, k, b, :],
                                                 start=(k == 0),
                                                 stop=(k == 2 * KE - 1))
                        # softmax via sigmoid: e^x = sig(x)/(1-sig(x))
                        NP = NBG * TC
                        sg = asb.tile([NP, S], F32, tag="sg", name="sg")
                        nc.scalar.activation(sg[:], sc[:], AF.Sigmoid, scale=SCALE)
                        # om = 1 - sg on the scalar engine (Identity table)
                        om = asb.tile([NP, S], F32, tag="om", name="om")
                        nc.scalar.activation(om[:], sg[:], AF.Identity,
                                             scale=-1.0, bias=1.0)
                        orc = asb.tile([NP, S], F32, tag="orc", name="orc")
                        nc.vector.reciprocal(orc[:], om[:])
                        e = asb.tile([NP, S], F32, tag="e", name="e")
                        nc.vector.tensor_tensor(e[:], sg[:], orc[:], ALU.mult)
                        den = asb.tile([NP, 1], F32, tag="den", name="den")
                        nc.vector.tensor_reduce(den[:], e[:], AX.X, ALU.add)
                        rec = asb.tile([NP, 1], F32, tag="rec", name="rec")
                        nc.vector.reciprocal(rec[:], den[:])
                        p_sb = asb.tile([NP, S], BF16, tag="p", name="p")
                        nc.vector.tensor_scalar_mul(p_sb[:], e[:], rec[:])
                        for j in range(NBG):
                            b = g * NBG + j
                            pt = aps.tile([S, TC], BF16, tag="a", name="pt",
                                          padded_shape=[128, TC])
                            nc.tensor.transpose(
                                pt[:], p_sb[j * TC:(j + 1) * TC, :],
                                ident[j * TC:(j + 1) * TC, j * TC:(j + 1) * TC])
                            pt_sb = asb.tile([S, TC], BF16, tag="pts", name="pts")
                            nc.scalar.copy(pt_sb[:], pt[:])
                            cx = aps.tile([128, KD * TC], F32, tag="a", name="cx")
                            for kd in range(KD):
                                nc.tensor.matmul(
                                    cx[:, kd * TC:(kd + 1) * TC],
                                    memN[:, b, kd * 128:(kd + 1) * 128],
                                    pt_sb[:], start=True, stop=True)
                            nc.vector.tensor_copy(
                                ctx_ch[c][:, :, b * TC:(b + 1) * TC],
                                cx[:].rearrange("p (k t) -> p k t", k=KD))
                    return fn

                def ffn_unit(c, m, half):
                    def fn():
                        Hc = Hchunks[c]
                        if m == 0 and half == 0:
                            mid_ch[c] = msb.tile([128, KD, BTC], BF16,
                                                 tag="mid", name="mid")
                        HW2 = BTC // 2
                        ps = mps.tile([128, HW2], F32, tag="md", name="md")
                        Hflat = Hc[:].rearrange("p k b t -> p (k b t)")
                        for k in range(KD):
                            lt = wf1_s[:, (k * KD + m) * 128:
                                       (k * KD + m + 1) * 128]
                            nc.tensor.matmul(ps[:], lt,
                                             Hflat[:, k * BTC + half * HW2:
                                                   k * BTC + (half + 1) * HW2],
                                             start=(k == 0), stop=False)
                        ctxf = ctx_ch[c][:].rearrange("p k t -> p (k t)")
                        for k in range(KD):
                            kk = KD + k
                            lt = wf1_s[:, (kk * KD + m) * 128:
                                       (kk * KD + m + 1) * 128]
                            nc.tensor.matmul(ps[:], lt,
                                             ctxf[:, k * BTC + half * HW2:
                                                  k * BTC + (half + 1) * HW2],
                                             start=False, stop=(k == KD - 1))
                        ab = asb.tile([128, HW2], F32, tag="mab", name="mab")
                        nc.scalar.activation(ab[:], ps[:], AF.Abs,
                                             bias=b1a_s[:, m:m + 1])
                        idt = asb.tile([128, HW2], F32, tag="midt", name="midt")
                        nc.scalar.activation(idt[:], ps[:], AF.Identity,
                                             scale=0.55, bias=b1h_s[:, m:m + 1])
                        nc.vector.scalar_tensor_tensor(
                            mid_ch[c][:, m, half * HW2:(half + 1) * HW2],
                            ab[:], 0.45, idt[:], ALU.mult, ALU.add)
                    return fn

                def wf2_unit(vc):
                    def fn():
                        t = wfp.tile([128, KD, 500], BF16, tag="wf2c", name="wf2c")
                        nc.sync.dma_start(t[:],
                                          wf2_3d[:, :, vc * 500:(vc + 1) * 500])
                        wf2_cur[0] = t
                    return fn

                def vocab_unit(c, vc, grp):
                    def fn():
                        wt = wf2_cur[0]
                        midf = mid_ch[c][:].rearrange("p k t -> p (k t)")
                        ps = lps.tile([128, 500], F32, tag="lg", name="lg")
                        for k in range(KD):
                            lt = midf[:, k * BTC + grp * 128:
                                      k * BTC + (grp + 1) * 128]
                            nc.tensor.matmul(ps[:], lt, wt[:, k, :],
                                             start=(k == 0), stop=(k == KD - 1))
                        lo = lsb.tile([128, 500], BF16, tag="lo", name="lo")
                        if grp % 2 == 0:
                            nc.scalar.copy(lo[:], ps[:])
                        else:
                            nc.vector.tensor_copy(lo[:], ps[:])
                        nb = 128 // TC
                        b0 = grp * nb
                        for bl in range(nb):
                            nc.sync.dma_start(
                                out.ap()[b0 + bl, c * TC:(c + 1) * TC,
                                         vc * 500:(vc + 1) * 500],
                                lo[bl * TC:(bl + 1) * TC, :])
                    return fn

                def queue_chunk_work(c):
                    for g in range(B // NBG):
                        q_lo.append((c, attn_unit(c, g), 800))
                    for m in range(KD):
                        for half in range(2):
                            q_lo.append((c, ffn_unit(c, m, half), 1750))
                    for vc in range(NVC):
                        q_lo.append((c, wf2_unit(vc), 0))
                        for grp in range(BTC // 128):
                            q_lo.append((c, vocab_unit(c, vc, grp), 900))

                morder_d = (list(range(KD, 2 * KD)) + list(range(2 * KD, 3 * KD))
                            + list(range(0, KD)) + list(range(3 * KD, 4 * KD)))

                for t in range(T):
                    tci = t // TC
                    t_loc = t % TC
                    if t_loc == 0:
                        if tci >= 3:
                            drain(n_hi=0, pe_budget=0, upto_chunk=tci - 3)
                        Hchunks[tci] = hpool.tile([128, KD, B, TC], BF16,
                                                  tag="H", name="H")
                    if t % XCHD == 0 and next_xd < NCH_D:
                        q_hi.extend(emit_xproj_d(next_xd))
                        next_xd += 1

                    if phases >= 6:
                        drain(n_hi=2, pe_budget=PE_BUDGET)
                    else:
                        drain(n_hi=2, pe_budget=0)

                    xt = xch_d[t // XCHD]
                    ps = dgps.tile([128, MD * 32], F32, tag="gd", name="gd")
                    nc.tensor.matmul(ps[:], ident[:, :],
                                     xt[:, :, :, t % XCHD]
                                     .rearrange("p m b -> p (m b)"),
                                     start=True, stop=False,
                                     skip_group_check=True)
                    for mi, m in enumerate(morder_d):
                        for k in range(KD):
                            if t == 0:
                                rhs = h0d[:, k * B:(k + 1) * B]
                            else:
                                rhs = h_prev_d[:, k * B:(k + 1) * B]
                            lt = whh_d_s[:, (k * MD + m) * 128:
                                         (k * MD + m + 1) * 128]
                            nc.tensor.matmul(
                                ps[:, m * 32:(m + 1) * 32], lt, rhs,
                                start=False,
                                stop=(mi == len(morder_d) - 1 and k == KD - 1),
                                skip_group_check=True)
                    gfd = dgsb.tile([128, GWD], F32, tag="gf", name="gf")
                    ggd = dgsb.tile([128, GWD], F32, tag="gg", name="gg")
                    gid = dgsb.tile([128, GWD], F32, tag="gi", name="gi")
                    god = dgsb.tile([128, GWD], F32, tag="go", name="go")
                    t1d = dgsb.tile([128, GWD], F32, tag="t1", name="t1")
                    t2d = dgsb.tile([128, GWD], F32, tag="t2", name="t2")
                    tctd = dgsb.tile([128, GWD], F32, tag="tct", name="tct")
                    nc.scalar.activation(gfd[:], ps[:, GWD:2 * GWD], AF.Sigmoid,
                                         scale=GSC)
                    nc.scalar.activation(ggd[:], ps[:, 2 * GWD:3 * GWD], AF.Tanh,
                                         scale=GSC)
                    nc.scalar.activation(gid[:], ps[:, 0:GWD], AF.Sigmoid,
                                         scale=GSC)
                    nc.scalar.activation(god[:], ps[:, 3 * GWD:4 * GWD], AF.Sigmoid,
                                         scale=GSC)
                    nc.vector.tensor_tensor(t1d[:], gfd[:], c_d[:], ALU.mult)
                    nc.vector.tensor_tensor(t2d[:], gid[:], ggd[:], ALU.mult)
                    nc.vector.tensor_tensor(c_d[:], t1d[:], t2d[:], ALU.add)
                    nc.scalar.activation(tctd[:], c_d[:], AF.Tanh)
                    hn = dgsb.tile([128, KD * B], BF16, tag="hn", name="hn")
                    nc.vector.tensor_tensor(hn[:], god[:], tctd[:], ALU.mult)
                    h_prev_d = hn
                    nc.gpsimd.tensor_copy(
                        Hchunks[tci][:, :, :, t_loc],
                        hn[:].rearrange("p (k b) -> p k b", k=KD))

                    if phases >= 5 and t_loc == TC - 1:
                        queue_chunk_work(tci)

                if phases >= 5:
                    while q_hi:
                        q_hi.pop(0)()
                    while q_lo:
                        q_lo.pop(0)[1]()
        except _Stop:
            pass

    nc.compile()
    return nc


def prep_inputs(i, S=128, T=128, V=32000, VS=4000):
    """Host-side input staging -> list of 8 per-core in_maps."""
    def as_np(x, dt=np.float32):
        return np.ascontiguousarray(np.asarray(x), dtype=dt)

    tok = as_np(i["tok_emb"]).astype(bf16)

    def idx_prep(flat):
        a = flat.astype(np.int16).reshape(-1, 16).T
        return np.ascontiguousarray(np.tile(a, (8, 1)))

    inp = as_np(i["inp"], np.int64)
    x = as_np(i["x"], np.int64)
    idx_e = idx_prep(inp.reshape(-1))                # b-major: b*S+s
    dmat = np.zeros((B, T), np.int64)
    dmat[:, 1:] = x[:, :T - 1]
    idx_d = idx_prep(dmat.reshape(-1))               # b-major: b*T+t

    startT = as_np(i["start_emb"]).reshape(D_EMB, 1).astype(bf16)

    est = as_np(i["enc_style_emb"])                  # [2, 512]
    diff_e = (est[1] - est[0]).reshape(1, -1).astype(bf16)
    e0T = np.ascontiguousarray(est[0].reshape(KD, 128).T)
    sty = as_np(i["style_emb"])                      # [2, 512]
    diff_s = (sty[1] - sty[0]).reshape(1, -1).astype(bf16)
    s0T = np.ascontiguousarray(sty[0].reshape(KD, 128).T)
    lab_i = as_np(i["label_i"], np.float32).reshape(1, B).astype(bf16)
    lab_d = as_np(i["label"], np.float32).reshape(1, B).astype(bf16)

    def wihT(w, nm):
        a = w.reshape(nm, 128, 128)
        return np.ascontiguousarray(a.transpose(2, 0, 1).reshape(128, nm * 128)
                                    ).astype(bf16)

    def whhT(w, nk, nm):
        a = w.reshape(nm, 128, nk, 128)
        a = a.transpose(3, 2, 0, 1)
        return np.ascontiguousarray(a.reshape(128, nk * nm * 128)).astype(bf16)

    # encoder m-chunk order permuted to (i, f, o, g) to match gate regions
    PERM_E = [0, 1, 2, 3, 6, 7, 4, 5]
    def permute_chunks(w, nm, perm):
        a = w.reshape(128, nm, 128)
        return np.ascontiguousarray(a[:, perm, :].reshape(128, nm * 128))
    wih_f = permute_chunks(wihT(as_np(i["Wih_f"]), ME), ME, PERM_E)
    wih_b = permute_chunks(wihT(as_np(i["Wih_b"]), ME), ME, PERM_E)
    wih_d = wihT(as_np(i["Wih_d"]), MD)
    if FP8:
        fp8 = ml_dtypes.float8_e4m3fn
        whh_f = (whhT(as_np(i["Whh_f"]), KE, ME).astype(np.float32) * 64).astype(fp8)
        whh_b = (whhT(as_np(i["Whh_b"]), KE, ME).astype(np.float32) * 64).astype(fp8)
        whh_d = (whhT(as_np(i["Whh_d"]), KD, MD).astype(np.float32) * 64).astype(fp8)
    else:
        whh_f = whhT(as_np(i["Whh_f"]), KE, ME)
        whh_b = whhT(as_np(i["Whh_b"]), KE, ME)
        whh_d = whhT(as_np(i["Whh_d"]), KD, MD)
    wtr = whhT(as_np(i["W_tr"]), KD, KD)
    wf1 = whhT(as_np(i["W_f1"]), 8, KD)

    wf2_full = as_np(i["W_f2"])                      # [V, 512]

    def bcol(v, nm):
        return np.ascontiguousarray(v.reshape(nm, 128).T)

    _xs = 64.0 if FP8 else 1.0
    bs_f = bcol(as_np(i["bih_f"]) + as_np(i["bhh_f"]), ME)[:, PERM_E] * _xs
    bs_b = bcol(as_np(i["bih_b"]) + as_np(i["bhh_b"]), ME)[:, PERM_E] * _xs
    bs_d = bcol(as_np(i["bih_d"]) + as_np(i["bhh_d"]), MD) * _xs
    b1 = as_np(i["b_f1"])
    b1a = bcol(b1, KD)
    b1h = bcol(0.55 * b1, KD)

    common = dict(tokb=tok, idx_e=idx_e, idx_d=idx_d, startT=startT,
                  diff_e=diff_e, e0T=e0T, lab_i=lab_i,
                  diff_s=diff_s, s0T=s0T, lab_d=lab_d,
                  wih_f=wih_f, wih_b=wih_b, wih_d=wih_d,
                  whh_f=whh_f, whh_b=whh_b, whh_d=whh_d,
                  wtr=wtr, wf1=wf1,
                  bs_f=bs_f, bs_b=bs_b, bs_d=bs_d, b1a=b1a, b1h=b1h)
    in_maps = []
    for c in range(N_CORES):
        shard = wf2_full[c * VS:(c + 1) * VS]        # [VS, 512]
        a = shard.reshape(VS, KD, 128)
        wf2c = np.ascontiguousarray(a.transpose(2, 1, 0).reshape(128, KD * VS)
                                    ).astype(bf16)
        in_maps.append(dict(common, wf2=wf2c))
    return in_maps


_NC_CACHE = {}


def kernel(**inputs):
    key = "full"
    if key not in _NC_CACHE:
        _NC_CACHE[key] = build()
    nc = _NC_CACHE[key]
    in_maps = prep_inputs(inputs)
    res = run_bass_kernel_spmd(nc, in_maps, core_ids=list(range(N_CORES)))
    full = np.concatenate([r["out"] for r in res.results], axis=2)
    return full.astype(np.float32)
